# revision 10
# baseline (speedup 1.0000x reference)
"""Trainium2 Bass kernel for nn_BatchAugmentation_53850299957920.

Reference pipeline (3 augs of x[2048, 20000]):
  same-class/cross-dataset mixup -> dropout -> per-row scale -> per-gene scale
  -> + 0.01*gaussian noise -> relu; concat augs (plus repeated ctx/y/cov
  metadata outputs).

Work split:
  * Host (exact jax-CPU threefry — RNG must match the reference bit-exactly
    for every discrete decision): all random draws, the [B,B] mixup candidate
    search, per-row/per-gene scalars.  The O(B*D) random tensors ship
    compactly: keep*gscale as f16, noise as int8 + per-aug dequant scale.
  * Device (8 NeuronCores, data-parallel over batch rows): all heavy [B,D]
    elementwise math:

      psum = Sp @ xq_tile        [+ diag(c1) @ xb_tile on even iters]   (PE)
      sx   = psum [+ c1*xb on odd iters, via scalar_tensor_tensor]      (DVE)
      sx  *= gd'                                                        (DVE)
      sx  += nz_i8                                       (DVE/GPSIMD alternating)
      out  = relu((0.01*qs) * sx)                                       (ACT)

    c1 = 100*scale*lam_eff, Sp[k,p] = 100*scale*(1-lam) for parent slot k ->
    row p, gd' = keep*gscale/qs (f16), nz_i8 = noise quantized to int8 with
    per-aug scale qs.  Folding 1/qs into gd and qs into the relu's
    per-partition scale makes the noise add a plain integer tensor_tensor.
    xq holds up to 128 deduplicated parent rows per 128-row tile (shared
    across the 3 augs); rare overflow rows are patched on the host.  The
    mix/noise work alternates between engines so no single engine gates the
    DMA-bound pipeline.
"""
import sys

if "/opt/trn_rl_repo" not in sys.path:
    sys.path.insert(0, "/opt/trn_rl_repo")

import numpy as np

N_AUG = 3
DROPOUT = 0.2
DS_MIN = 0.8
DS_MAX = 1.2
MIXUP_P = 0.3
ALPHA = 0.4
GENE_P = 0.2

N_CORES = 8
P = 128
B, D = 2048, 20000
R = B // N_CORES          # rows per core
T = R // P                # row-tiles per core
W = 2000                  # column chunk
BANK_F32 = 512            # one PSUM bank in f32 elements

_PROGRAM_CACHE = {}


# ----------------------------------------------------------------------------
# host side: exact RNG decomposition (jax CPU threefry == reference bits)
# ----------------------------------------------------------------------------
def _host_decompose(x, ctx, y):
    import jax
    import jax.numpy as jnp

    cpu = jax.devices("cpu")[0]
    Bx, Dx = x.shape
    base_key = jax.device_put(jax.random.key(42), cpu)

    c1 = np.empty((N_AUG, Bx), np.float32)
    c2 = np.empty((N_AUG, Bx), np.float32)
    j_sel_all = np.empty((N_AUG, Bx), np.int64)
    do_mix_all = np.empty((N_AUG, Bx), bool)
    ctxs = []
    keep_list = []
    gscale_list = []
    noise_list = []
    qs = np.empty((N_AUG,), np.float32)

    yy = np.asarray(y)
    cc = np.asarray(ctx)

    with jax.default_device(cpu):
        for a in range(N_AUG):
            k = jax.random.fold_in(base_key, a)
            k_mix, k_drop, k_scale, k_gmask, k_gscale, k_noise = \
                jax.random.split(k, 6)

            # --- mixup selection (bit-exact vs reference) ---
            k_sel, k_pick, k_lam, k_coin = jax.random.split(k_mix, 4)
            mix_mask = np.asarray(jax.random.uniform(k_sel, (Bx,))) < MIXUP_P
            cand = (yy[None, :] == yy[:, None]) & (cc[None, :] != cc[:, None])
            counts = cand.sum(axis=1).astype(np.int32)
            u = np.asarray(jax.random.uniform(k_pick, (Bx,)))
            kth = np.minimum(
                np.floor(u * counts.astype(np.float32)).astype(np.int32),
                np.maximum(counts - 1, 0),
            )
            csum = np.cumsum(cand.astype(np.int32), axis=1)
            j_sel = np.argmax(csum > kth[:, None], axis=1)
            lam = np.asarray(
                jax.random.beta(k_lam, ALPHA, ALPHA, (Bx,), dtype=jnp.float32)
            )
            do_mix = mix_mask & (counts > 0)
            coin = np.asarray(jax.random.uniform(k_coin, (Bx,))) < 0.5
            ctx_new = np.where(do_mix & (~coin), cc[j_sel], cc).astype(cc.dtype)

            scale = (
                np.asarray(jax.random.uniform(k_scale, (Bx, 1)))[:, 0]
                * (DS_MAX - DS_MIN) + DS_MIN
            ).astype(np.float32)
            lam_eff = np.where(do_mix, lam, np.float32(1.0)).astype(np.float32)
            c1[a] = 100.0 * scale * lam_eff
            c2[a] = np.where(do_mix, 100.0 * scale * (1.0 - lam_eff), 0.0)
            j_sel_all[a] = j_sel
            do_mix_all[a] = do_mix
            ctxs.append(ctx_new)

            gmask = np.asarray(jax.random.uniform(k_gmask, (Dx,))) < GENE_P
            gs_draw = np.asarray(jax.random.uniform(k_gscale, (Dx,))) * 0.4 + 0.8
            gscale_list.append(np.where(gmask, gs_draw, 1.0).astype(np.float32))

            # big exact draws
            keep_list.append(
                np.asarray(jax.random.uniform(k_drop, (Bx, Dx))) > DROPOUT
            )
            n = np.asarray(
                jax.random.normal(k_noise, (Bx, Dx), dtype=jnp.float32)
            )
            qs[a] = np.float32(np.abs(n).max() / 127.0)
            noise_list.append(
                np.clip(np.rint(n / qs[a]), -127, 127).astype(np.int8)
            )

    return dict(c1=c1, c2=c2, j_sel=j_sel_all, do_mix=do_mix_all,
                ctxs=np.stack(ctxs, axis=0), keep=keep_list,
                gscale=gscale_list, noise_i8=noise_list, qs=qs)


# ----------------------------------------------------------------------------
# per-core input assembly
# ----------------------------------------------------------------------------
def _build_in_maps(x, dec):
    """Returns (in_maps, overflow) where overflow is a list of
    (a, global_row) whose parent didn't fit in the 128-slot cap."""
    qs_full = np.ascontiguousarray(
        np.broadcast_to((np.float32(0.01) * dec["qs"])[:, None, None],
                        (N_AUG, P, 1)).astype(np.float32))
    in_maps = []
    overflow = []
    for c in range(N_CORES):
        rows = slice(c * R, (c + 1) * R)
        gd = np.empty((N_AUG, R, D), np.float16)
        nz = np.empty((N_AUG, R, D), np.int8)
        for a in range(N_AUG):
            gd[a] = dec["keep"][a][rows] * \
                (dec["gscale"][a] / dec["qs"][a])[None, :]
            nz[a] = dec["noise_i8"][a][rows]

        xq = np.zeros((T, P, D), np.float32)
        diag = np.zeros((N_AUG, T, P, P), np.float32)
        sp = np.zeros((N_AUG, T, P, P), np.float32)
        ar = np.arange(P)
        for t in range(T):
            slot_of = {}          # parent row j -> slot
            slot_js = []
            for a in range(N_AUG):
                base = c * R + t * P
                diag[a, t, ar, ar] = dec["c1"][a, base:base + P]
                mix_p = np.nonzero(dec["do_mix"][a, base:base + P])[0]
                for p in mix_p:
                    j = int(dec["j_sel"][a, base + p])
                    k = slot_of.get(j)
                    if k is None:
                        if len(slot_js) >= P:
                            overflow.append((a, base + p))
                            continue
                        k = len(slot_js)
                        slot_of[j] = k
                        slot_js.append(j)
                    sp[a, t, k, p] = dec["c2"][a, base + p]
            if slot_js:
                xq[t, :len(slot_js)] = x[np.asarray(slot_js)]

        in_maps.append({
            "xb": x[rows], "xq": xq, "gd": gd, "nz": nz,
            "c1": np.ascontiguousarray(
                dec["c1"][:, rows].reshape(N_AUG, T, P, 1)),
            "diag": diag, "sp": sp, "qs": qs_full,
        })
    return in_maps, overflow


def _patch_overflow(counts, x, dec, overflow):
    """Host-exact recompute of rows whose mixup parent had no xq slot."""
    for a, i in overflow:
        gd_row = (dec["keep"][a][i] * dec["gscale"][a]).astype(np.float16)
        s = dec["c1"][a, i] * x[i] + dec["c2"][a, i] * x[dec["j_sel"][a, i]]
        h = gd_row.astype(np.float32) * s + \
            dec["qs"][a] * dec["noise_i8"][a][i].astype(np.float32)
        counts[a * B + i] = np.maximum(np.float32(0.01) * h, 0.0)


# ----------------------------------------------------------------------------
# device program (v3)
# ----------------------------------------------------------------------------
def _build_program():
    import concourse.bacc as bacc
    from concourse import mybir
    from concourse.tile import TileContext

    A = N_AUG
    nc = bacc.Bacc("TRN2")
    f32, f16, i8 = mybir.dt.float32, mybir.dt.float16, mybir.dt.int8
    t_xb = nc.dram_tensor("xb", [R, D], f32, kind="ExternalInput")
    t_xq = nc.dram_tensor("xq", [T, P, D], f32, kind="ExternalInput")
    t_gd = nc.dram_tensor("gd", [A, R, D], f16, kind="ExternalInput")
    t_nz = nc.dram_tensor("nz", [A, R, D], i8, kind="ExternalInput")
    t_c1 = nc.dram_tensor("c1", [A, T, P, 1], f32, kind="ExternalInput")
    t_diag = nc.dram_tensor("diag", [A, T, P, P], f32, kind="ExternalInput")
    t_sp = nc.dram_tensor("sp", [A, T, P, P], f32, kind="ExternalInput")
    t_qs = nc.dram_tensor("qs", [A, P, 1], f32, kind="ExternalInput")
    t_out = nc.dram_tensor("out", [A, R, D], f32, kind="ExternalOutput")

    Relu = mybir.ActivationFunctionType.Relu
    mult = mybir.AluOpType.mult
    add = mybir.AluOpType.add
    NCH = D // W
    NSUB = (W + BANK_F32 - 1) // BANK_F32
    WPAD = NSUB * BANK_F32

    with TileContext(nc) as tc:
        with (
            tc.tile_pool(name="wts", bufs=1) as wt_pool,
            tc.tile_pool(name="io", bufs=3) as io_pool,
            tc.tile_pool(name="io16", bufs=5) as io16_pool,
            tc.tile_pool(name="work", bufs=5) as work_pool,
            tc.tile_pool(name="psum", bufs=2, space="PSUM") as psum_pool,
        ):
            sp_sb, c1_sb, diag_sb = [], [], []
            for a in range(A):
                for t in range(T):
                    spt = wt_pool.tile([P, P], f32, name=f"sp_{a}_{t}")
                    nc.sync.dma_start(spt[:, :], t_sp[a, t])
                    dgt = wt_pool.tile([P, P], f32, name=f"diag_{a}_{t}")
                    nc.sync.dma_start(dgt[:, :], t_diag[a, t])
                    c1t = wt_pool.tile([P, 1], f32, name=f"c1_{a}_{t}")
                    nc.sync.dma_start(c1t[:, :], t_c1[a, t])
                    sp_sb.append(spt)
                    diag_sb.append(dgt)
                    c1_sb.append(c1t)
            qs_sb = []
            for a in range(A):
                qst = wt_pool.tile([P, 1], f32, name=f"qs_{a}")
                nc.sync.dma_start(qst[:, :], t_qs[a])
                qs_sb.append(qst)

            it = 0
            for t in range(T):
                rows = slice(t * P, (t + 1) * P)
                for ch in range(NCH):
                    cols = slice(ch * W, (ch + 1) * W)
                    xb_c = io_pool.tile([P, W], f32, name="xb_c", bufs=4)
                    nc.sync.dma_start(xb_c[:, :], t_xb[rows, cols])
                    xq_c = io_pool.tile([P, W], f32, name="xq_c", bufs=3)
                    nc.sync.dma_start(xq_c[:, :], t_xq[t, :, cols])
                    for a in range(A):
                        gd_c = io16_pool.tile([P, W], f16, name="gd_c")
                        nc.sync.dma_start(gd_c[:, :], t_gd[a, rows, cols])
                        nz_c = io16_pool.tile([P, W], i8, name="nz_c")
                        nc.sync.dma_start(nz_c[:, :], t_nz[a, rows, cols])

                        mix_on_pe = (it % 2 == 0)
                        noise_on_gpsimd = (it % 2 == 1)
                        it += 1

                        ps = psum_pool.tile([P, WPAD], f32, name="ps")
                        if mix_on_pe:
                            for s in range(NSUB):
                                lo = s * BANK_F32
                                hi = min(W, lo + BANK_F32)
                                nc.tensor.matmul(
                                    ps[:, lo:hi], diag_sb[a * T + t][:, :],
                                    xb_c[:, lo:hi], start=True, stop=False)
                            for s in range(NSUB):
                                lo = s * BANK_F32
                                hi = min(W, lo + BANK_F32)
                                nc.tensor.matmul(
                                    ps[:, lo:hi], sp_sb[a * T + t][:, :],
                                    xq_c[:, lo:hi], start=False, stop=True)
                        else:
                            for s in range(NSUB):
                                lo = s * BANK_F32
                                hi = min(W, lo + BANK_F32)
                                nc.tensor.matmul(
                                    ps[:, lo:hi], sp_sb[a * T + t][:, :],
                                    xq_c[:, lo:hi], start=True, stop=True)

                        sx = work_pool.tile([P, W], f32, name="sx")
                        if mix_on_pe:
                            # sx = psum * gd
                            nc.vector.tensor_tensor(
                                sx[:, :], ps[:, 0:W], gd_c[:, :], mult)
                        else:
                            # sx = (c1*xb + psum) then *= gd
                            nc.vector.scalar_tensor_tensor(
                                sx[:, :], xb_c[:, :], c1_sb[a * T + t][:, :],
                                ps[:, 0:W], mult, add)
                            nc.vector.tensor_tensor(
                                sx[:, :], sx[:, :], gd_c[:, :], mult)
                        # sx += nz_i8 (dequant folded into gd and relu scale)
                        if noise_on_gpsimd:
                            nf = work_pool.tile([P, W], f32, name="nf",
                                                bufs=3)
                            nc.gpsimd.tensor_copy(nf[:, :], nz_c[:, :])
                            nc.gpsimd.tensor_tensor(sx[:, :], sx[:, :],
                                                    nf[:, :], add)
                        else:
                            nc.vector.tensor_tensor(sx[:, :], sx[:, :],
                                                    nz_c[:, :], add)
                        out_c = io_pool.tile([P, W], f32, name="out_c", bufs=3)
                        nc.scalar.activation(out_c[:, :], sx[:, :], Relu,
                                             scale=qs_sb[a][:, :])
                        nc.sync.dma_start(t_out[a, rows, cols], out_c[:, :])
    nc.finalize()
    return nc


def _get_program():
    key = (R, D, W, "v3")
    if key not in _PROGRAM_CACHE:
        _PROGRAM_CACHE[key] = _build_program()
    return _PROGRAM_CACHE[key]


# ----------------------------------------------------------------------------
# entry point
# ----------------------------------------------------------------------------
def kernel(x, ctx, y, cont_covs, cat_covs):
    from concourse.bass_utils import run_bass_kernel_spmd

    x = np.ascontiguousarray(np.asarray(x, dtype=np.float32))
    ctx = np.asarray(ctx)
    y = np.asarray(y)
    assert x.shape == (B, D), x.shape

    dec = _host_decompose(x, ctx, y)
    in_maps, overflow = _build_in_maps(x, dec)

    nc = _get_program()
    res = run_bass_kernel_spmd(nc, in_maps, core_ids=list(range(N_CORES)))

    counts = np.empty((N_AUG * B, D), np.float32)
    for c in range(N_CORES):
        out_c = res.results[c]["out"]
        for a in range(N_AUG):
            counts[a * B + c * R:(a * B + (c + 1) * R)] = out_c[a]
    _patch_overflow(counts, x, dec, overflow)

    aug_ctxs = dec["ctxs"].reshape(-1)
    y_rep = np.tile(y, N_AUG)
    cont_rep = np.tile(np.asarray(cont_covs), (N_AUG, 1)).reshape(-1)
    cat_rep = np.tile(np.asarray(cat_covs), (N_AUG, 1)).reshape(-1)
    return counts, aug_ctxs, y_rep, cont_rep, cat_rep


# revision 11
# speedup vs baseline: 1.1293x; 1.1293x over previous
"""Trainium2 Bass kernel for nn_BatchAugmentation_53850299957920.

Reference pipeline (3 augs of x[2048, 20000]):
  same-class/cross-dataset mixup -> dropout -> per-row scale -> per-gene scale
  -> + 0.01*gaussian noise -> relu; concat augs (plus repeated ctx/y/cov
  metadata outputs).

Work split:
  * Host (exact jax-CPU threefry — RNG must match the reference bit-exactly
    for every discrete decision): all random draws, the [B,B] mixup candidate
    search, per-row/per-gene scalars.  The O(B*D) random tensors ship
    compactly: keep*gscale as f16, noise as int8 + per-aug dequant scale.
  * Device (8 NeuronCores, data-parallel over batch rows): all heavy [B,D]
    elementwise math:

      psum = Sp @ xq_tile                                               (PE)
      sx   = c1*xb + psum        (scalar_tensor_tensor)                 (DVE)
      sx  *= gd'                                                        (DVE)
      sx  += nz_i8                                                   (GPSIMD)
      out  = relu((0.01*qs) * sx)                                       (ACT)

    c1 = 100*scale*lam_eff, Sp[k,p] = 100*scale*(1-lam) for parent slot k ->
    row p, gd' = keep*gscale/qs (f16), nz_i8 = noise quantized to int8 with
    per-aug scale qs.  Folding 1/qs into gd and qs into the relu's
    per-partition scale makes the noise add a plain integer tensor_tensor.
    xq holds up to 128 deduplicated parent rows per 128-row tile (shared
    across the 3 augs); rare overflow rows are patched on the host.  The
    mix/noise work alternates between engines so no single engine gates the
    DMA-bound pipeline.
"""
import sys

if "/opt/trn_rl_repo" not in sys.path:
    sys.path.insert(0, "/opt/trn_rl_repo")

import numpy as np

N_AUG = 3
DROPOUT = 0.2
DS_MIN = 0.8
DS_MAX = 1.2
MIXUP_P = 0.3
ALPHA = 0.4
GENE_P = 0.2

N_CORES = 8
P = 128
B, D = 2048, 20000
R = B // N_CORES          # rows per core
T = R // P                # row-tiles per core
W = 1000                  # column chunk
BANK_F32 = 512            # one PSUM bank in f32 elements

_PROGRAM_CACHE = {}


# ----------------------------------------------------------------------------
# host side: exact RNG decomposition (jax CPU threefry == reference bits)
# ----------------------------------------------------------------------------
def _host_decompose(x, ctx, y):
    import jax
    import jax.numpy as jnp

    cpu = jax.devices("cpu")[0]
    Bx, Dx = x.shape
    base_key = jax.device_put(jax.random.key(42), cpu)

    c1 = np.empty((N_AUG, Bx), np.float32)
    c2 = np.empty((N_AUG, Bx), np.float32)
    j_sel_all = np.empty((N_AUG, Bx), np.int64)
    do_mix_all = np.empty((N_AUG, Bx), bool)
    ctxs = []
    keep_list = []
    gscale_list = []
    noise_list = []
    qs = np.empty((N_AUG,), np.float32)

    yy = np.asarray(y)
    cc = np.asarray(ctx)

    with jax.default_device(cpu):
        for a in range(N_AUG):
            k = jax.random.fold_in(base_key, a)
            k_mix, k_drop, k_scale, k_gmask, k_gscale, k_noise = \
                jax.random.split(k, 6)

            # --- mixup selection (bit-exact vs reference) ---
            k_sel, k_pick, k_lam, k_coin = jax.random.split(k_mix, 4)
            mix_mask = np.asarray(jax.random.uniform(k_sel, (Bx,))) < MIXUP_P
            cand = (yy[None, :] == yy[:, None]) & (cc[None, :] != cc[:, None])
            counts = cand.sum(axis=1).astype(np.int32)
            u = np.asarray(jax.random.uniform(k_pick, (Bx,)))
            kth = np.minimum(
                np.floor(u * counts.astype(np.float32)).astype(np.int32),
                np.maximum(counts - 1, 0),
            )
            csum = np.cumsum(cand.astype(np.int32), axis=1)
            j_sel = np.argmax(csum > kth[:, None], axis=1)
            lam = np.asarray(
                jax.random.beta(k_lam, ALPHA, ALPHA, (Bx,), dtype=jnp.float32)
            )
            do_mix = mix_mask & (counts > 0)
            coin = np.asarray(jax.random.uniform(k_coin, (Bx,))) < 0.5
            ctx_new = np.where(do_mix & (~coin), cc[j_sel], cc).astype(cc.dtype)

            scale = (
                np.asarray(jax.random.uniform(k_scale, (Bx, 1)))[:, 0]
                * (DS_MAX - DS_MIN) + DS_MIN
            ).astype(np.float32)
            lam_eff = np.where(do_mix, lam, np.float32(1.0)).astype(np.float32)
            c1[a] = 100.0 * scale * lam_eff
            c2[a] = np.where(do_mix, 100.0 * scale * (1.0 - lam_eff), 0.0)
            j_sel_all[a] = j_sel
            do_mix_all[a] = do_mix
            ctxs.append(ctx_new)

            gmask = np.asarray(jax.random.uniform(k_gmask, (Dx,))) < GENE_P
            gs_draw = np.asarray(jax.random.uniform(k_gscale, (Dx,))) * 0.4 + 0.8
            gscale_list.append(np.where(gmask, gs_draw, 1.0).astype(np.float32))

            # big exact draws
            keep_list.append(
                np.asarray(jax.random.uniform(k_drop, (Bx, Dx))) > DROPOUT
            )
            n = np.asarray(
                jax.random.normal(k_noise, (Bx, Dx), dtype=jnp.float32)
            )
            qs[a] = np.float32(np.abs(n).max() / 127.0)
            noise_list.append(
                np.clip(np.rint(n / qs[a]), -127, 127).astype(np.int8)
            )

    return dict(c1=c1, c2=c2, j_sel=j_sel_all, do_mix=do_mix_all,
                ctxs=np.stack(ctxs, axis=0), keep=keep_list,
                gscale=gscale_list, noise_i8=noise_list, qs=qs)


# ----------------------------------------------------------------------------
# per-core input assembly
# ----------------------------------------------------------------------------
def _build_in_maps(x, dec):
    """Returns (in_maps, overflow) where overflow is a list of
    (a, global_row) whose parent didn't fit in the 128-slot cap."""
    qs_full = np.ascontiguousarray(
        np.broadcast_to((np.float32(0.01) * dec["qs"])[:, None, None],
                        (N_AUG, P, 1)).astype(np.float32))
    in_maps = []
    overflow = []
    for c in range(N_CORES):
        rows = slice(c * R, (c + 1) * R)
        gd = np.empty((N_AUG, R, D), np.float16)
        nz = np.empty((N_AUG, R, D), np.int8)
        for a in range(N_AUG):
            gd[a] = dec["keep"][a][rows] * \
                (dec["gscale"][a] / dec["qs"][a])[None, :]
            nz[a] = dec["noise_i8"][a][rows]

        xq = np.zeros((T, P, D), np.float32)
        sp = np.zeros((N_AUG, T, P, P), np.float32)
        for t in range(T):
            slot_of = {}          # parent row j -> slot
            slot_js = []
            for a in range(N_AUG):
                base = c * R + t * P
                mix_p = np.nonzero(dec["do_mix"][a, base:base + P])[0]
                for p in mix_p:
                    j = int(dec["j_sel"][a, base + p])
                    k = slot_of.get(j)
                    if k is None:
                        if len(slot_js) >= P:
                            overflow.append((a, base + p))
                            continue
                        k = len(slot_js)
                        slot_of[j] = k
                        slot_js.append(j)
                    sp[a, t, k, p] = dec["c2"][a, base + p]
            if slot_js:
                xq[t, :len(slot_js)] = x[np.asarray(slot_js)]

        in_maps.append({
            "xb": x[rows], "xq": xq, "gd": gd, "nz": nz,
            "c1": np.ascontiguousarray(
                dec["c1"][:, rows].reshape(N_AUG, T, P, 1)),
            "sp": sp, "qs": qs_full,
        })
    return in_maps, overflow


def _patch_overflow(counts, x, dec, overflow):
    """Host-exact recompute of rows whose mixup parent had no xq slot."""
    for a, i in overflow:
        gd_row = (dec["keep"][a][i] * dec["gscale"][a]).astype(np.float16)
        s = dec["c1"][a, i] * x[i] + dec["c2"][a, i] * x[dec["j_sel"][a, i]]
        h = gd_row.astype(np.float32) * s + \
            dec["qs"][a] * dec["noise_i8"][a][i].astype(np.float32)
        counts[a * B + i] = np.maximum(np.float32(0.01) * h, 0.0)


# ----------------------------------------------------------------------------
# device program (v3)
# ----------------------------------------------------------------------------
def _build_program():
    import concourse.bacc as bacc
    from concourse import mybir
    from concourse.tile import TileContext

    A = N_AUG
    nc = bacc.Bacc("TRN2")
    f32, f16, i8 = mybir.dt.float32, mybir.dt.float16, mybir.dt.int8
    t_xb = nc.dram_tensor("xb", [R, D], f32, kind="ExternalInput")
    t_xq = nc.dram_tensor("xq", [T, P, D], f32, kind="ExternalInput")
    t_gd = nc.dram_tensor("gd", [A, R, D], f16, kind="ExternalInput")
    t_nz = nc.dram_tensor("nz", [A, R, D], i8, kind="ExternalInput")
    t_c1 = nc.dram_tensor("c1", [A, T, P, 1], f32, kind="ExternalInput")
    t_sp = nc.dram_tensor("sp", [A, T, P, P], f32, kind="ExternalInput")
    t_qs = nc.dram_tensor("qs", [A, P, 1], f32, kind="ExternalInput")
    t_out = nc.dram_tensor("out", [A, R, D], f32, kind="ExternalOutput")

    Relu = mybir.ActivationFunctionType.Relu
    mult = mybir.AluOpType.mult
    add = mybir.AluOpType.add
    NCH = D // W
    NSUB = (W + BANK_F32 - 1) // BANK_F32

    with TileContext(nc) as tc:
        with (
            tc.tile_pool(name="wts", bufs=1) as wt_pool,
            tc.tile_pool(name="io", bufs=5) as io_pool,
            tc.tile_pool(name="io16", bufs=6) as io16_pool,
            tc.tile_pool(name="work", bufs=6) as work_pool,
            tc.tile_pool(name="psum", bufs=4, space="PSUM") as psum_pool,
        ):
            sp_sb, c1_sb = [], []
            for a in range(A):
                for t in range(T):
                    spt = wt_pool.tile([P, P], f32, name=f"sp_{a}_{t}")
                    nc.sync.dma_start(spt[:, :], t_sp[a, t])
                    c1t = wt_pool.tile([P, 1], f32, name=f"c1_{a}_{t}")
                    nc.sync.dma_start(c1t[:, :], t_c1[a, t])
                    sp_sb.append(spt)
                    c1_sb.append(c1t)
            qs_sb = []
            for a in range(A):
                qst = wt_pool.tile([P, 1], f32, name=f"qs_{a}")
                nc.sync.dma_start(qst[:, :], t_qs[a])
                qs_sb.append(qst)

            for t in range(T):
                rows = slice(t * P, (t + 1) * P)
                for ch in range(NCH):
                    cols = slice(ch * W, (ch + 1) * W)
                    xb_c = io_pool.tile([P, W], f32, name="xb_c")
                    nc.sync.dma_start(xb_c[:, :], t_xb[rows, cols])
                    xq_c = io_pool.tile([P, W], f32, name="xq_c")
                    nc.sync.dma_start(xq_c[:, :], t_xq[t, :, cols])
                    for a in range(A):
                        gd_c = io16_pool.tile([P, W], f16, name="gd_c")
                        nc.sync.dma_start(gd_c[:, :], t_gd[a, rows, cols])
                        nz_c = io16_pool.tile([P, W], i8, name="nz_c")
                        nc.sync.dma_start(nz_c[:, :], t_nz[a, rows, cols])

                        ps = psum_pool.tile([P, W], f32, name="ps")
                        for s in range(NSUB):
                            lo = s * BANK_F32
                            hi = min(W, lo + BANK_F32)
                            nc.tensor.matmul(
                                ps[:, lo:hi], sp_sb[a * T + t][:, :],
                                xq_c[:, lo:hi], start=True, stop=True)

                        sx = work_pool.tile([P, W], f32, name="sx")
                        # sx = c1*xb + psum(parent part)
                        nc.vector.scalar_tensor_tensor(
                            sx[:, :], xb_c[:, :], c1_sb[a * T + t][:, :],
                            ps[:, 0:W], mult, add)
                        # sx *= gd
                        nc.vector.tensor_tensor(sx[:, :], sx[:, :],
                                                gd_c[:, :], mult)
                        # sx += nz_i8 (dequant folded into gd and relu scale)
                        nc.gpsimd.tensor_tensor(sx[:, :], sx[:, :],
                                                nz_c[:, :], add)
                        out_c = io_pool.tile([P, W], f32, name="out_c")
                        nc.scalar.activation(out_c[:, :], sx[:, :], Relu,
                                             scale=qs_sb[a][:, :])
                        nc.sync.dma_start(t_out[a, rows, cols], out_c[:, :])
    nc.finalize()
    return nc


def _get_program():
    key = (R, D, W, "v4")
    if key not in _PROGRAM_CACHE:
        _PROGRAM_CACHE[key] = _build_program()
    return _PROGRAM_CACHE[key]


# ----------------------------------------------------------------------------
# entry point
# ----------------------------------------------------------------------------
def kernel(x, ctx, y, cont_covs, cat_covs):
    from concourse.bass_utils import run_bass_kernel_spmd

    x = np.ascontiguousarray(np.asarray(x, dtype=np.float32))
    ctx = np.asarray(ctx)
    y = np.asarray(y)
    assert x.shape == (B, D), x.shape

    dec = _host_decompose(x, ctx, y)
    in_maps, overflow = _build_in_maps(x, dec)

    nc = _get_program()
    res = run_bass_kernel_spmd(nc, in_maps, core_ids=list(range(N_CORES)))

    counts = np.empty((N_AUG * B, D), np.float32)
    for c in range(N_CORES):
        out_c = res.results[c]["out"]
        for a in range(N_AUG):
            counts[a * B + c * R:(a * B + (c + 1) * R)] = out_c[a]
    _patch_overflow(counts, x, dec, overflow)

    aug_ctxs = dec["ctxs"].reshape(-1)
    y_rep = np.tile(y, N_AUG)
    cont_rep = np.tile(np.asarray(cont_covs), (N_AUG, 1)).reshape(-1)
    cat_rep = np.tile(np.asarray(cat_covs), (N_AUG, 1)).reshape(-1)
    return counts, aug_ctxs, y_rep, cont_rep, cat_rep


# revision 12
# speedup vs baseline: 1.4210x; 1.2582x over previous
"""Trainium2 Bass kernel for nn_BatchAugmentation_53850299957920.

Reference pipeline (3 augs of x[2048, 20000]):
  same-class/cross-dataset mixup -> dropout -> per-row scale -> per-gene scale
  -> + 0.01*gaussian noise -> relu; concat augs (plus repeated ctx/y/cov
  metadata outputs).

Work split:
  * Host (exact jax-CPU threefry — RNG must match the reference bit-exactly
    for every discrete decision): all random draws, the [B,B] mixup candidate
    search, per-row/per-gene scalars.  The O(B*D) random tensors ship
    compactly: keep*gscale as f16, noise as int8 + per-aug dequant scale.
  * Device (8 NeuronCores, data-parallel over batch rows): all heavy [B,D]
    elementwise math:

      psum = diag(c1) @ xb_tile + Sp @ xq_tile                          (PE)
      sx   = psum * gd'                                                 (DVE)
      sx  += nz_i8                                                   (GPSIMD)
      out  = relu((0.01*qs) * sx)                                       (ACT)

    c1 = 100*scale*lam_eff, Sp[k,p] = 100*scale*(1-lam) for parent slot k ->
    row p, gd' = keep*gscale/qs (f16), nz_i8 = noise quantized to int8 with
    per-aug scale qs.  Folding 1/qs into gd and qs into the relu's
    per-partition scale makes the noise add a plain integer tensor_tensor.
    xq holds up to 128 deduplicated parent rows per 128-row tile (shared
    across the 3 augs); rare overflow rows are patched on the host.  The
    mix/noise work alternates between engines so no single engine gates the
    DMA-bound pipeline.
"""
import sys

if "/opt/trn_rl_repo" not in sys.path:
    sys.path.insert(0, "/opt/trn_rl_repo")

import numpy as np

N_AUG = 3
DROPOUT = 0.2
DS_MIN = 0.8
DS_MAX = 1.2
MIXUP_P = 0.3
ALPHA = 0.4
GENE_P = 0.2

N_CORES = 8
P = 128
B, D = 2048, 20000
R = B // N_CORES          # rows per core
T = R // P                # row-tiles per core
W = 2000                  # column chunk
BANK_F32 = 512            # one PSUM bank in f32 elements

_PROGRAM_CACHE = {}


# ----------------------------------------------------------------------------
# host side: exact RNG decomposition (jax CPU threefry == reference bits)
# ----------------------------------------------------------------------------
def _host_decompose(x, ctx, y):
    import jax
    import jax.numpy as jnp

    cpu = jax.devices("cpu")[0]
    Bx, Dx = x.shape
    base_key = jax.device_put(jax.random.key(42), cpu)

    c1 = np.empty((N_AUG, Bx), np.float32)
    c2 = np.empty((N_AUG, Bx), np.float32)
    j_sel_all = np.empty((N_AUG, Bx), np.int64)
    do_mix_all = np.empty((N_AUG, Bx), bool)
    ctxs = []
    keep_list = []
    gscale_list = []
    noise_list = []
    qs = np.empty((N_AUG,), np.float32)

    yy = np.asarray(y)
    cc = np.asarray(ctx)

    with jax.default_device(cpu):
        for a in range(N_AUG):
            k = jax.random.fold_in(base_key, a)
            k_mix, k_drop, k_scale, k_gmask, k_gscale, k_noise = \
                jax.random.split(k, 6)

            # --- mixup selection (bit-exact vs reference) ---
            k_sel, k_pick, k_lam, k_coin = jax.random.split(k_mix, 4)
            mix_mask = np.asarray(jax.random.uniform(k_sel, (Bx,))) < MIXUP_P
            cand = (yy[None, :] == yy[:, None]) & (cc[None, :] != cc[:, None])
            counts = cand.sum(axis=1).astype(np.int32)
            u = np.asarray(jax.random.uniform(k_pick, (Bx,)))
            kth = np.minimum(
                np.floor(u * counts.astype(np.float32)).astype(np.int32),
                np.maximum(counts - 1, 0),
            )
            csum = np.cumsum(cand.astype(np.int32), axis=1)
            j_sel = np.argmax(csum > kth[:, None], axis=1)
            lam = np.asarray(
                jax.random.beta(k_lam, ALPHA, ALPHA, (Bx,), dtype=jnp.float32)
            )
            do_mix = mix_mask & (counts > 0)
            coin = np.asarray(jax.random.uniform(k_coin, (Bx,))) < 0.5
            ctx_new = np.where(do_mix & (~coin), cc[j_sel], cc).astype(cc.dtype)

            scale = (
                np.asarray(jax.random.uniform(k_scale, (Bx, 1)))[:, 0]
                * (DS_MAX - DS_MIN) + DS_MIN
            ).astype(np.float32)
            lam_eff = np.where(do_mix, lam, np.float32(1.0)).astype(np.float32)
            c1[a] = 100.0 * scale * lam_eff
            c2[a] = np.where(do_mix, 100.0 * scale * (1.0 - lam_eff), 0.0)
            j_sel_all[a] = j_sel
            do_mix_all[a] = do_mix
            ctxs.append(ctx_new)

            gmask = np.asarray(jax.random.uniform(k_gmask, (Dx,))) < GENE_P
            gs_draw = np.asarray(jax.random.uniform(k_gscale, (Dx,))) * 0.4 + 0.8
            gscale_list.append(np.where(gmask, gs_draw, 1.0).astype(np.float32))

            # big exact draws
            keep_list.append(
                np.asarray(jax.random.uniform(k_drop, (Bx, Dx))) > DROPOUT
            )
            n = np.asarray(
                jax.random.normal(k_noise, (Bx, Dx), dtype=jnp.float32)
            )
            qs[a] = np.float32(np.abs(n).max() / 127.0)
            noise_list.append(
                np.clip(np.rint(n / qs[a]), -127, 127).astype(np.int8)
            )

    return dict(c1=c1, c2=c2, j_sel=j_sel_all, do_mix=do_mix_all,
                ctxs=np.stack(ctxs, axis=0), keep=keep_list,
                gscale=gscale_list, noise_i8=noise_list, qs=qs)


# ----------------------------------------------------------------------------
# per-core input assembly
# ----------------------------------------------------------------------------
def _build_in_maps(x, dec):
    """Returns (in_maps, overflow) where overflow is a list of
    (a, global_row) whose parent didn't fit in the 128-slot cap."""
    qs_full = np.ascontiguousarray(
        np.broadcast_to((np.float32(0.01) * dec["qs"])[:, None, None],
                        (N_AUG, P, 1)).astype(np.float32))
    in_maps = []
    overflow = []
    for c in range(N_CORES):
        rows = slice(c * R, (c + 1) * R)
        gd = np.empty((N_AUG, R, D), np.float16)
        nz = np.empty((N_AUG, R, D), np.int8)
        for a in range(N_AUG):
            gd[a] = dec["keep"][a][rows] * \
                (dec["gscale"][a] / dec["qs"][a])[None, :]
            nz[a] = dec["noise_i8"][a][rows]

        xq = np.zeros((T, P, D), np.float32)
        diag = np.zeros((N_AUG, T, P, P), np.float32)
        sp = np.zeros((N_AUG, T, P, P), np.float32)
        ar = np.arange(P)
        for t in range(T):
            slot_of = {}          # parent row j -> slot
            slot_js = []
            for a in range(N_AUG):
                base = c * R + t * P
                diag[a, t, ar, ar] = dec["c1"][a, base:base + P]
                mix_p = np.nonzero(dec["do_mix"][a, base:base + P])[0]
                for p in mix_p:
                    j = int(dec["j_sel"][a, base + p])
                    k = slot_of.get(j)
                    if k is None:
                        if len(slot_js) >= P:
                            overflow.append((a, base + p))
                            continue
                        k = len(slot_js)
                        slot_of[j] = k
                        slot_js.append(j)
                    sp[a, t, k, p] = dec["c2"][a, base + p]
            if slot_js:
                xq[t, :len(slot_js)] = x[np.asarray(slot_js)]

        in_maps.append({
            "xb": x[rows], "xq": xq, "gd": gd, "nz": nz,
            "diag": diag, "sp": sp, "qs": qs_full,
        })
    return in_maps, overflow


def _patch_overflow(counts, x, dec, overflow):
    """Host-exact recompute of rows whose mixup parent had no xq slot."""
    for a, i in overflow:
        gd_row = (dec["keep"][a][i] * dec["gscale"][a]).astype(np.float16)
        s = dec["c1"][a, i] * x[i] + dec["c2"][a, i] * x[dec["j_sel"][a, i]]
        h = gd_row.astype(np.float32) * s + \
            dec["qs"][a] * dec["noise_i8"][a][i].astype(np.float32)
        counts[a * B + i] = np.maximum(np.float32(0.01) * h, 0.0)


# ----------------------------------------------------------------------------
# device program (v3)
# ----------------------------------------------------------------------------
def _build_program():
    import concourse.bacc as bacc
    from concourse import mybir
    from concourse.tile import TileContext

    A = N_AUG
    nc = bacc.Bacc("TRN2")
    f32, f16, i8 = mybir.dt.float32, mybir.dt.float16, mybir.dt.int8
    t_xb = nc.dram_tensor("xb", [R, D], f32, kind="ExternalInput")
    t_xq = nc.dram_tensor("xq", [T, P, D], f32, kind="ExternalInput")
    t_gd = nc.dram_tensor("gd", [A, R, D], f16, kind="ExternalInput")
    t_nz = nc.dram_tensor("nz", [A, R, D], i8, kind="ExternalInput")
    t_diag = nc.dram_tensor("diag", [A, T, P, P], f32, kind="ExternalInput")
    t_sp = nc.dram_tensor("sp", [A, T, P, P], f32, kind="ExternalInput")
    t_qs = nc.dram_tensor("qs", [A, P, 1], f32, kind="ExternalInput")
    t_out = nc.dram_tensor("out", [A, R, D], f32, kind="ExternalOutput")

    Relu = mybir.ActivationFunctionType.Relu
    mult = mybir.AluOpType.mult
    add = mybir.AluOpType.add
    NCH = D // W
    NSUB = (W + BANK_F32 - 1) // BANK_F32
    WPAD = NSUB * BANK_F32

    with TileContext(nc) as tc:
        with (
            tc.tile_pool(name="wts", bufs=1) as wt_pool,
            tc.tile_pool(name="io", bufs=3) as io_pool,
            tc.tile_pool(name="io16", bufs=6) as io16_pool,
            tc.tile_pool(name="work", bufs=4) as work_pool,
            tc.tile_pool(name="psum", bufs=2, space="PSUM") as psum_pool,
        ):
            sp_sb, diag_sb = [], []
            for a in range(A):
                for t in range(T):
                    spt = wt_pool.tile([P, P], f32, name=f"sp_{a}_{t}")
                    nc.sync.dma_start(spt[:, :], t_sp[a, t])
                    dgt = wt_pool.tile([P, P], f32, name=f"diag_{a}_{t}")
                    nc.sync.dma_start(dgt[:, :], t_diag[a, t])
                    sp_sb.append(spt)
                    diag_sb.append(dgt)
            qs_sb = []
            for a in range(A):
                qst = wt_pool.tile([P, 1], f32, name=f"qs_{a}")
                nc.sync.dma_start(qst[:, :], t_qs[a])
                qs_sb.append(qst)

            for t in range(T):
                rows = slice(t * P, (t + 1) * P)
                for ch in range(NCH):
                    cols = slice(ch * W, (ch + 1) * W)
                    xb_c = io_pool.tile([P, W], f32, name="xb_c", bufs=4)
                    nc.sync.dma_start(xb_c[:, :], t_xb[rows, cols])
                    xq_c = io_pool.tile([P, W], f32, name="xq_c", bufs=3)
                    nc.sync.dma_start(xq_c[:, :], t_xq[t, :, cols])
                    for a in range(A):
                        gd_c = io16_pool.tile([P, W], f16, name="gd_c")
                        nc.sync.dma_start(gd_c[:, :], t_gd[a, rows, cols])
                        nz_c = io16_pool.tile([P, W], i8, name="nz_c")
                        nc.sync.dma_start(nz_c[:, :], t_nz[a, rows, cols])

                        ps = psum_pool.tile([P, WPAD], f32, name="ps")
                        for s in range(NSUB):
                            lo = s * BANK_F32
                            hi = min(W, lo + BANK_F32)
                            nc.tensor.matmul(
                                ps[:, lo:hi], diag_sb[a * T + t][:, :],
                                xb_c[:, lo:hi], start=True, stop=False)
                        for s in range(NSUB):
                            lo = s * BANK_F32
                            hi = min(W, lo + BANK_F32)
                            nc.tensor.matmul(
                                ps[:, lo:hi], sp_sb[a * T + t][:, :],
                                xq_c[:, lo:hi], start=False, stop=True)

                        sx = work_pool.tile([P, W], f32, name="sx")
                        # sx = psum * gd
                        nc.vector.tensor_tensor(sx[:, :], ps[:, 0:W],
                                                gd_c[:, :], mult)
                        # sx += nz_i8 (dequant folded into gd and relu scale)
                        nc.gpsimd.tensor_tensor(sx[:, :], sx[:, :],
                                                nz_c[:, :], add)
                        out_c = io_pool.tile([P, W], f32, name="out_c", bufs=3)
                        nc.scalar.activation(out_c[:, :], sx[:, :], Relu,
                                             scale=qs_sb[a][:, :])
                        nc.scalar.dma_start(t_out[a, rows, cols], out_c[:, :])
    nc.finalize()
    return nc


def _get_program():
    key = (R, D, W, "v5")
    if key not in _PROGRAM_CACHE:
        _PROGRAM_CACHE[key] = _build_program()
    return _PROGRAM_CACHE[key]


# ----------------------------------------------------------------------------
# entry point
# ----------------------------------------------------------------------------
def kernel(x, ctx, y, cont_covs, cat_covs):
    from concourse.bass_utils import run_bass_kernel_spmd

    x = np.ascontiguousarray(np.asarray(x, dtype=np.float32))
    ctx = np.asarray(ctx)
    y = np.asarray(y)
    assert x.shape == (B, D), x.shape

    dec = _host_decompose(x, ctx, y)
    in_maps, overflow = _build_in_maps(x, dec)

    nc = _get_program()
    res = run_bass_kernel_spmd(nc, in_maps, core_ids=list(range(N_CORES)))

    counts = np.empty((N_AUG * B, D), np.float32)
    for c in range(N_CORES):
        out_c = res.results[c]["out"]
        for a in range(N_AUG):
            counts[a * B + c * R:(a * B + (c + 1) * R)] = out_c[a]
    _patch_overflow(counts, x, dec, overflow)

    aug_ctxs = dec["ctxs"].reshape(-1)
    y_rep = np.tile(y, N_AUG)
    cont_rep = np.tile(np.asarray(cont_covs), (N_AUG, 1)).reshape(-1)
    cat_rep = np.tile(np.asarray(cat_covs), (N_AUG, 1)).reshape(-1)
    return counts, aug_ctxs, y_rep, cont_rep, cat_rep


# revision 13
# speedup vs baseline: 1.5000x; 1.0556x over previous
"""Trainium2 Bass kernel for nn_BatchAugmentation_53850299957920.

Reference pipeline (3 augs of x[2048, 20000]):
  same-class/cross-dataset mixup -> dropout -> per-row scale -> per-gene scale
  -> + 0.01*gaussian noise -> relu; concat augs (plus repeated ctx/y/cov
  metadata outputs).

Work split:
  * Host (exact jax-CPU threefry — RNG must match the reference bit-exactly
    for every discrete decision): all random draws, the [B,B] mixup candidate
    search, per-row/per-gene scalars.  The O(B*D) random tensors ship
    compactly: keep*gscale as f16, noise as int8 + per-aug dequant scale.
  * Device (8 NeuronCores, data-parallel over batch rows): all heavy [B,D]
    elementwise math:

      psum = diag(c1) @ xb_tile + Sp @ xq_tile                          (PE)
      sx   = psum * gd'                                                 (DVE)
      sx  += nz_i8                                                   (GPSIMD)
      out  = relu((0.01*qs) * sx)                                       (ACT)

    c1 = 100*scale*lam_eff, Sp[k,p] = 100*scale*(1-lam) for parent slot k ->
    row p, gd' = keep*gscale/qs (f16), nz_i8 = noise quantized to int8 with
    per-aug scale qs.  Folding 1/qs into gd and qs into the relu's
    per-partition scale makes the noise add a plain integer tensor_tensor.
    xq holds up to 128 deduplicated parent rows per 128-row tile (shared
    across the 3 augs); rare overflow rows are patched on the host.  The
    mix/noise work alternates between engines so no single engine gates the
    DMA-bound pipeline.
"""
import sys

if "/opt/trn_rl_repo" not in sys.path:
    sys.path.insert(0, "/opt/trn_rl_repo")

import numpy as np

N_AUG = 3
DROPOUT = 0.2
DS_MIN = 0.8
DS_MAX = 1.2
MIXUP_P = 0.3
ALPHA = 0.4
GENE_P = 0.2

N_CORES = 8
P = 128
B, D = 2048, 20000
R = B // N_CORES          # rows per core
T = R // P                # row-tiles per core
W = 2000                  # column chunk
BANK_F32 = 512            # one PSUM bank in f32 elements

_PROGRAM_CACHE = {}


# ----------------------------------------------------------------------------
# host side: exact RNG decomposition (jax CPU threefry == reference bits)
# ----------------------------------------------------------------------------
def _host_decompose(x, ctx, y):
    import jax
    import jax.numpy as jnp

    cpu = jax.devices("cpu")[0]
    Bx, Dx = x.shape
    base_key = jax.device_put(jax.random.key(42), cpu)

    c1 = np.empty((N_AUG, Bx), np.float32)
    c2 = np.empty((N_AUG, Bx), np.float32)
    j_sel_all = np.empty((N_AUG, Bx), np.int64)
    do_mix_all = np.empty((N_AUG, Bx), bool)
    ctxs = []
    keep_list = []
    gscale_list = []
    noise_list = []
    qs = np.empty((N_AUG,), np.float32)

    yy = np.asarray(y)
    cc = np.asarray(ctx)

    with jax.default_device(cpu):
        for a in range(N_AUG):
            k = jax.random.fold_in(base_key, a)
            k_mix, k_drop, k_scale, k_gmask, k_gscale, k_noise = \
                jax.random.split(k, 6)

            # --- mixup selection (bit-exact vs reference) ---
            k_sel, k_pick, k_lam, k_coin = jax.random.split(k_mix, 4)
            mix_mask = np.asarray(jax.random.uniform(k_sel, (Bx,))) < MIXUP_P
            cand = (yy[None, :] == yy[:, None]) & (cc[None, :] != cc[:, None])
            counts = cand.sum(axis=1).astype(np.int32)
            u = np.asarray(jax.random.uniform(k_pick, (Bx,)))
            kth = np.minimum(
                np.floor(u * counts.astype(np.float32)).astype(np.int32),
                np.maximum(counts - 1, 0),
            )
            csum = np.cumsum(cand.astype(np.int32), axis=1)
            j_sel = np.argmax(csum > kth[:, None], axis=1)
            lam = np.asarray(
                jax.random.beta(k_lam, ALPHA, ALPHA, (Bx,), dtype=jnp.float32)
            )
            do_mix = mix_mask & (counts > 0)
            coin = np.asarray(jax.random.uniform(k_coin, (Bx,))) < 0.5
            ctx_new = np.where(do_mix & (~coin), cc[j_sel], cc).astype(cc.dtype)

            scale = (
                np.asarray(jax.random.uniform(k_scale, (Bx, 1)))[:, 0]
                * (DS_MAX - DS_MIN) + DS_MIN
            ).astype(np.float32)
            lam_eff = np.where(do_mix, lam, np.float32(1.0)).astype(np.float32)
            c1[a] = 100.0 * scale * lam_eff
            c2[a] = np.where(do_mix, 100.0 * scale * (1.0 - lam_eff), 0.0)
            j_sel_all[a] = j_sel
            do_mix_all[a] = do_mix
            ctxs.append(ctx_new)

            gmask = np.asarray(jax.random.uniform(k_gmask, (Dx,))) < GENE_P
            gs_draw = np.asarray(jax.random.uniform(k_gscale, (Dx,))) * 0.4 + 0.8
            gscale_list.append(np.where(gmask, gs_draw, 1.0).astype(np.float32))

            # big exact draws
            keep_list.append(
                np.asarray(jax.random.uniform(k_drop, (Bx, Dx))) > DROPOUT
            )
            n = np.asarray(
                jax.random.normal(k_noise, (Bx, Dx), dtype=jnp.float32)
            )
            qs[a] = np.float32(np.abs(n).max() / 127.0)
            noise_list.append(
                np.clip(np.rint(n / qs[a]), -127, 127).astype(np.int8)
            )

    return dict(c1=c1, c2=c2, j_sel=j_sel_all, do_mix=do_mix_all,
                ctxs=np.stack(ctxs, axis=0), keep=keep_list,
                gscale=gscale_list, noise_i8=noise_list, qs=qs)


# ----------------------------------------------------------------------------
# per-core input assembly
# ----------------------------------------------------------------------------
def _build_in_maps(x, dec):
    """Returns (in_maps, overflow) where overflow is a list of
    (a, global_row) whose parent didn't fit in the 128-slot cap."""
    qs_full = np.ascontiguousarray(
        np.broadcast_to((np.float32(0.01) * dec["qs"])[:, None, None],
                        (N_AUG, P, 1)).astype(np.float32))
    in_maps = []
    overflow = []
    for c in range(N_CORES):
        rows = slice(c * R, (c + 1) * R)
        gd = np.empty((N_AUG, R, D), np.float16)
        nz = np.empty((N_AUG, R, D), np.int8)
        for a in range(N_AUG):
            gd[a] = dec["keep"][a][rows] * \
                (dec["gscale"][a] / dec["qs"][a])[None, :]
            nz[a] = dec["noise_i8"][a][rows]

        xq = np.zeros((T, P, D), np.float32)
        diag = np.zeros((N_AUG, T, P, P), np.float32)
        sp = np.zeros((N_AUG, T, P, P), np.float32)
        ar = np.arange(P)
        for t in range(T):
            slot_of = {}          # parent row j -> slot
            slot_js = []
            for a in range(N_AUG):
                base = c * R + t * P
                diag[a, t, ar, ar] = dec["c1"][a, base:base + P]
                mix_p = np.nonzero(dec["do_mix"][a, base:base + P])[0]
                for p in mix_p:
                    j = int(dec["j_sel"][a, base + p])
                    k = slot_of.get(j)
                    if k is None:
                        if len(slot_js) >= P:
                            overflow.append((a, base + p))
                            continue
                        k = len(slot_js)
                        slot_of[j] = k
                        slot_js.append(j)
                    sp[a, t, k, p] = dec["c2"][a, base + p]
            if slot_js:
                xq[t, :len(slot_js)] = x[np.asarray(slot_js)]

        in_maps.append({
            "xb": x[rows], "xq": xq, "gd": gd, "nz": nz,
            "c1": np.ascontiguousarray(
                dec["c1"][:, rows].reshape(N_AUG, T, P, 1)),
            "diag": diag, "sp": sp, "qs": qs_full,
        })
    return in_maps, overflow


def _patch_overflow(counts, x, dec, overflow):
    """Host-exact recompute of rows whose mixup parent had no xq slot."""
    for a, i in overflow:
        gd_row = (dec["keep"][a][i] * dec["gscale"][a]).astype(np.float16)
        s = dec["c1"][a, i] * x[i] + dec["c2"][a, i] * x[dec["j_sel"][a, i]]
        h = gd_row.astype(np.float32) * s + \
            dec["qs"][a] * dec["noise_i8"][a][i].astype(np.float32)
        counts[a * B + i] = np.maximum(np.float32(0.01) * h, 0.0)


# ----------------------------------------------------------------------------
# device program (v3)
# ----------------------------------------------------------------------------
def _build_program():
    import concourse.bacc as bacc
    from concourse import mybir
    from concourse.tile import TileContext

    A = N_AUG
    nc = bacc.Bacc("TRN2")
    f32, f16, i8 = mybir.dt.float32, mybir.dt.float16, mybir.dt.int8
    t_xb = nc.dram_tensor("xb", [R, D], f32, kind="ExternalInput")
    t_xq = nc.dram_tensor("xq", [T, P, D], f32, kind="ExternalInput")
    t_gd = nc.dram_tensor("gd", [A, R, D], f16, kind="ExternalInput")
    t_nz = nc.dram_tensor("nz", [A, R, D], i8, kind="ExternalInput")
    t_c1 = nc.dram_tensor("c1", [A, T, P, 1], f32, kind="ExternalInput")
    t_diag = nc.dram_tensor("diag", [A, T, P, P], f32, kind="ExternalInput")
    t_sp = nc.dram_tensor("sp", [A, T, P, P], f32, kind="ExternalInput")
    t_qs = nc.dram_tensor("qs", [A, P, 1], f32, kind="ExternalInput")
    t_out = nc.dram_tensor("out", [A, R, D], f32, kind="ExternalOutput")

    Relu = mybir.ActivationFunctionType.Relu
    mult = mybir.AluOpType.mult
    add = mybir.AluOpType.add
    NCH = D // W
    NSUB = (W + BANK_F32 - 1) // BANK_F32
    WPAD = NSUB * BANK_F32

    with TileContext(nc) as tc:
        with (
            tc.tile_pool(name="wts", bufs=1) as wt_pool,
            tc.tile_pool(name="io", bufs=3) as io_pool,
            tc.tile_pool(name="io16", bufs=6) as io16_pool,
            tc.tile_pool(name="work", bufs=4) as work_pool,
            tc.tile_pool(name="psum", bufs=2, space="PSUM") as psum_pool,
        ):
            sp_sb, diag_sb, c1_sb = [], [], []
            for a in range(A):
                for t in range(T):
                    spt = wt_pool.tile([P, P], f32, name=f"sp_{a}_{t}")
                    nc.sync.dma_start(spt[:, :], t_sp[a, t])
                    dgt = wt_pool.tile([P, P], f32, name=f"diag_{a}_{t}")
                    nc.sync.dma_start(dgt[:, :], t_diag[a, t])
                    c1t = wt_pool.tile([P, 1], f32, name=f"c1_{a}_{t}")
                    nc.sync.dma_start(c1t[:, :], t_c1[a, t])
                    sp_sb.append(spt)
                    diag_sb.append(dgt)
                    c1_sb.append(c1t)
            qs_sb = []
            for a in range(A):
                qst = wt_pool.tile([P, 1], f32, name=f"qs_{a}")
                nc.sync.dma_start(qst[:, :], t_qs[a])
                qs_sb.append(qst)

            it = 0
            for t in range(T):
                rows = slice(t * P, (t + 1) * P)
                for ch in range(NCH):
                    cols = slice(ch * W, (ch + 1) * W)
                    xb_c = io_pool.tile([P, W], f32, name="xb_c", bufs=4)
                    nc.sync.dma_start(xb_c[:, :], t_xb[rows, cols])
                    xq_c = io_pool.tile([P, W], f32, name="xq_c", bufs=3)
                    nc.sync.dma_start(xq_c[:, :], t_xq[t, :, cols])
                    for a in range(A):
                        gd_c = io16_pool.tile([P, W], f16, name="gd_c")
                        nc.sync.dma_start(gd_c[:, :], t_gd[a, rows, cols])
                        nz_c = io16_pool.tile([P, W], i8, name="nz_c")
                        nc.sync.dma_start(nz_c[:, :], t_nz[a, rows, cols])

                        mix_on_pe = (it % 2 == 0)
                        it += 1
                        ps = psum_pool.tile([P, WPAD], f32, name="ps")
                        if mix_on_pe:
                            for s in range(NSUB):
                                lo = s * BANK_F32
                                hi = min(W, lo + BANK_F32)
                                nc.tensor.matmul(
                                    ps[:, lo:hi], diag_sb[a * T + t][:, :],
                                    xb_c[:, lo:hi], start=True, stop=False)
                            for s in range(NSUB):
                                lo = s * BANK_F32
                                hi = min(W, lo + BANK_F32)
                                nc.tensor.matmul(
                                    ps[:, lo:hi], sp_sb[a * T + t][:, :],
                                    xq_c[:, lo:hi], start=False, stop=True)
                        else:
                            for s in range(NSUB):
                                lo = s * BANK_F32
                                hi = min(W, lo + BANK_F32)
                                nc.tensor.matmul(
                                    ps[:, lo:hi], sp_sb[a * T + t][:, :],
                                    xq_c[:, lo:hi], start=True, stop=True)

                        sx = work_pool.tile([P, W], f32, name="sx")
                        if mix_on_pe:
                            # sx = psum * gd
                            nc.vector.tensor_tensor(sx[:, :], ps[:, 0:W],
                                                    gd_c[:, :], mult)
                        else:
                            # sx = c1*xb + psum, then *= gd
                            nc.vector.scalar_tensor_tensor(
                                sx[:, :], xb_c[:, :], c1_sb[a * T + t][:, :],
                                ps[:, 0:W], mult, add)
                            nc.vector.tensor_tensor(sx[:, :], sx[:, :],
                                                    gd_c[:, :], mult)
                        # sx += nz_i8 (dequant folded into gd and relu scale)
                        nc.gpsimd.tensor_tensor(sx[:, :], sx[:, :],
                                                nz_c[:, :], add)
                        out_c = io_pool.tile([P, W], f32, name="out_c", bufs=3)
                        nc.scalar.activation(out_c[:, :], sx[:, :], Relu,
                                             scale=qs_sb[a][:, :])
                        nc.scalar.dma_start(t_out[a, rows, cols], out_c[:, :])
    nc.finalize()
    return nc


def _get_program():
    key = (R, D, W, "v6")
    if key not in _PROGRAM_CACHE:
        _PROGRAM_CACHE[key] = _build_program()
    return _PROGRAM_CACHE[key]


# ----------------------------------------------------------------------------
# entry point
# ----------------------------------------------------------------------------
def kernel(x, ctx, y, cont_covs, cat_covs):
    from concourse.bass_utils import run_bass_kernel_spmd

    x = np.ascontiguousarray(np.asarray(x, dtype=np.float32))
    ctx = np.asarray(ctx)
    y = np.asarray(y)
    assert x.shape == (B, D), x.shape

    dec = _host_decompose(x, ctx, y)
    in_maps, overflow = _build_in_maps(x, dec)

    nc = _get_program()
    res = run_bass_kernel_spmd(nc, in_maps, core_ids=list(range(N_CORES)))

    counts = np.empty((N_AUG * B, D), np.float32)
    for c in range(N_CORES):
        out_c = res.results[c]["out"]
        for a in range(N_AUG):
            counts[a * B + c * R:(a * B + (c + 1) * R)] = out_c[a]
    _patch_overflow(counts, x, dec, overflow)

    aug_ctxs = dec["ctxs"].reshape(-1)
    y_rep = np.tile(y, N_AUG)
    cont_rep = np.tile(np.asarray(cont_covs), (N_AUG, 1)).reshape(-1)
    cat_rep = np.tile(np.asarray(cat_covs), (N_AUG, 1)).reshape(-1)
    return counts, aug_ctxs, y_rep, cont_rep, cat_rep


# revision 14
# speedup vs baseline: 1.5171x; 1.0114x over previous
"""Trainium2 Bass kernel for nn_BatchAugmentation_53850299957920.

Reference pipeline (3 augs of x[2048, 20000]):
  same-class/cross-dataset mixup -> dropout -> per-row scale -> per-gene scale
  -> + 0.01*gaussian noise -> relu; concat augs (plus repeated ctx/y/cov
  metadata outputs).

Work split:
  * Host (exact jax-CPU threefry — RNG must match the reference bit-exactly
    for every discrete decision): all random draws, the [B,B] mixup candidate
    search, per-row/per-gene scalars.  The O(B*D) random tensors ship
    compactly: keep*gscale as f16, noise as int8 + per-aug dequant scale.
  * Device (8 NeuronCores, data-parallel over batch rows): all heavy [B,D]
    elementwise math:

      psum = diag(c1) @ xb_tile + Sp @ xq_tile                          (PE)
      sx   = psum * gd'                                                 (DVE)
      sx  += nz_i8                                                   (GPSIMD)
      out  = relu((0.01*qs) * sx)                                       (ACT)

    c1 = 100*scale*lam_eff, Sp[k,p] = 100*scale*(1-lam) for parent slot k ->
    row p, gd' = keep*gscale/qs (f16), nz_i8 = noise quantized to int8 with
    per-aug scale qs.  Folding 1/qs into gd and qs into the relu's
    per-partition scale makes the noise add a plain integer tensor_tensor.
    xq holds up to 128 deduplicated parent rows per 128-row tile (shared
    across the 3 augs); rare overflow rows are patched on the host.  The
    mix/noise work alternates between engines so no single engine gates the
    DMA-bound pipeline.
"""
import sys

if "/opt/trn_rl_repo" not in sys.path:
    sys.path.insert(0, "/opt/trn_rl_repo")

import numpy as np

N_AUG = 3
DROPOUT = 0.2
DS_MIN = 0.8
DS_MAX = 1.2
MIXUP_P = 0.3
ALPHA = 0.4
GENE_P = 0.2

N_CORES = 8
P = 128
B, D = 2048, 20000
R = B // N_CORES          # rows per core
T = R // P                # row-tiles per core
W = 2000                  # column chunk
BANK_F32 = 512            # one PSUM bank in f32 elements

_PROGRAM_CACHE = {}


# ----------------------------------------------------------------------------
# host side: exact RNG decomposition (jax CPU threefry == reference bits)
# ----------------------------------------------------------------------------
def _host_decompose(x, ctx, y):
    import jax
    import jax.numpy as jnp

    cpu = jax.devices("cpu")[0]
    Bx, Dx = x.shape
    base_key = jax.device_put(jax.random.key(42), cpu)

    c1 = np.empty((N_AUG, Bx), np.float32)
    c2 = np.empty((N_AUG, Bx), np.float32)
    j_sel_all = np.empty((N_AUG, Bx), np.int64)
    do_mix_all = np.empty((N_AUG, Bx), bool)
    ctxs = []
    keep_list = []
    gscale_list = []
    noise_list = []
    qs = np.empty((N_AUG,), np.float32)

    yy = np.asarray(y)
    cc = np.asarray(ctx)

    with jax.default_device(cpu):
        for a in range(N_AUG):
            k = jax.random.fold_in(base_key, a)
            k_mix, k_drop, k_scale, k_gmask, k_gscale, k_noise = \
                jax.random.split(k, 6)

            # --- mixup selection (bit-exact vs reference) ---
            k_sel, k_pick, k_lam, k_coin = jax.random.split(k_mix, 4)
            mix_mask = np.asarray(jax.random.uniform(k_sel, (Bx,))) < MIXUP_P
            cand = (yy[None, :] == yy[:, None]) & (cc[None, :] != cc[:, None])
            counts = cand.sum(axis=1).astype(np.int32)
            u = np.asarray(jax.random.uniform(k_pick, (Bx,)))
            kth = np.minimum(
                np.floor(u * counts.astype(np.float32)).astype(np.int32),
                np.maximum(counts - 1, 0),
            )
            csum = np.cumsum(cand.astype(np.int32), axis=1)
            j_sel = np.argmax(csum > kth[:, None], axis=1)
            lam = np.asarray(
                jax.random.beta(k_lam, ALPHA, ALPHA, (Bx,), dtype=jnp.float32)
            )
            do_mix = mix_mask & (counts > 0)
            coin = np.asarray(jax.random.uniform(k_coin, (Bx,))) < 0.5
            ctx_new = np.where(do_mix & (~coin), cc[j_sel], cc).astype(cc.dtype)

            scale = (
                np.asarray(jax.random.uniform(k_scale, (Bx, 1)))[:, 0]
                * (DS_MAX - DS_MIN) + DS_MIN
            ).astype(np.float32)
            lam_eff = np.where(do_mix, lam, np.float32(1.0)).astype(np.float32)
            c1[a] = 100.0 * scale * lam_eff
            c2[a] = np.where(do_mix, 100.0 * scale * (1.0 - lam_eff), 0.0)
            j_sel_all[a] = j_sel
            do_mix_all[a] = do_mix
            ctxs.append(ctx_new)

            gmask = np.asarray(jax.random.uniform(k_gmask, (Dx,))) < GENE_P
            gs_draw = np.asarray(jax.random.uniform(k_gscale, (Dx,))) * 0.4 + 0.8
            gscale_list.append(np.where(gmask, gs_draw, 1.0).astype(np.float32))

            # big exact draws
            keep_list.append(
                np.asarray(jax.random.uniform(k_drop, (Bx, Dx))) > DROPOUT
            )
            n = np.asarray(
                jax.random.normal(k_noise, (Bx, Dx), dtype=jnp.float32)
            )
            qs[a] = np.float32(np.abs(n).max() / 127.0)
            noise_list.append(
                np.clip(np.rint(n / qs[a]), -127, 127).astype(np.int8)
            )

    return dict(c1=c1, c2=c2, j_sel=j_sel_all, do_mix=do_mix_all,
                ctxs=np.stack(ctxs, axis=0), keep=keep_list,
                gscale=gscale_list, noise_i8=noise_list, qs=qs)


# ----------------------------------------------------------------------------
# per-core input assembly
# ----------------------------------------------------------------------------
def _build_in_maps(x, dec):
    """Returns (in_maps, overflow) where overflow is a list of
    (a, global_row) whose parent didn't fit in the 128-slot cap."""
    qs_full = np.ascontiguousarray(
        np.broadcast_to((np.float32(0.01) * dec["qs"])[:, None, None],
                        (N_AUG, P, 1)).astype(np.float32))
    in_maps = []
    overflow = []
    NCH = D // W
    for c in range(N_CORES):
        rows = slice(c * R, (c + 1) * R)
        # packed per-chunk stream: [gd f16 bytes | nz i8 bytes] per chunk
        gn = np.empty((N_AUG, R, NCH, 3 * W), np.uint8)
        for a in range(N_AUG):
            gd_a = (dec["keep"][a][rows] *
                    (dec["gscale"][a] / dec["qs"][a])[None, :]).astype(
                        np.float16)
            gn[a, :, :, :2 * W] = gd_a.reshape(R, NCH, W).view(np.uint8)
            gn[a, :, :, 2 * W:] = \
                dec["noise_i8"][a][rows].reshape(R, NCH, W).view(np.uint8)

        xq = np.zeros((T, P, D), np.float32)
        diag = np.zeros((N_AUG, T, P, P), np.float32)
        sp = np.zeros((N_AUG, T, P, P), np.float32)
        ar = np.arange(P)
        for t in range(T):
            slot_of = {}          # parent row j -> slot
            slot_js = []
            for a in range(N_AUG):
                base = c * R + t * P
                diag[a, t, ar, ar] = dec["c1"][a, base:base + P]
                mix_p = np.nonzero(dec["do_mix"][a, base:base + P])[0]
                for p in mix_p:
                    j = int(dec["j_sel"][a, base + p])
                    k = slot_of.get(j)
                    if k is None:
                        if len(slot_js) >= P:
                            overflow.append((a, base + p))
                            continue
                        k = len(slot_js)
                        slot_of[j] = k
                        slot_js.append(j)
                    sp[a, t, k, p] = dec["c2"][a, base + p]
            if slot_js:
                xq[t, :len(slot_js)] = x[np.asarray(slot_js)]

        in_maps.append({
            "xb": x[rows], "xq": xq, "gn": gn,
            "c1": np.ascontiguousarray(
                dec["c1"][:, rows].reshape(N_AUG, T, P, 1)),
            "diag": diag, "sp": sp, "qs": qs_full,
        })
    return in_maps, overflow


def _patch_overflow(counts, x, dec, overflow):
    """Host-exact recompute of rows whose mixup parent had no xq slot."""
    for a, i in overflow:
        gd_row = (dec["keep"][a][i] * dec["gscale"][a]).astype(np.float16)
        s = dec["c1"][a, i] * x[i] + dec["c2"][a, i] * x[dec["j_sel"][a, i]]
        h = gd_row.astype(np.float32) * s + \
            dec["qs"][a] * dec["noise_i8"][a][i].astype(np.float32)
        counts[a * B + i] = np.maximum(np.float32(0.01) * h, 0.0)


# ----------------------------------------------------------------------------
# device program (v3)
# ----------------------------------------------------------------------------
def _build_program():
    import concourse.bacc as bacc
    from concourse import mybir
    from concourse.tile import TileContext

    A = N_AUG
    nc = bacc.Bacc("TRN2")
    f32, f16, i8 = mybir.dt.float32, mybir.dt.float16, mybir.dt.int8
    t_xb = nc.dram_tensor("xb", [R, D], f32, kind="ExternalInput")
    t_xq = nc.dram_tensor("xq", [T, P, D], f32, kind="ExternalInput")
    NCHd = D // W
    t_gn = nc.dram_tensor("gn", [A, R, NCHd, 3 * W], mybir.dt.uint8,
                          kind="ExternalInput")
    t_c1 = nc.dram_tensor("c1", [A, T, P, 1], f32, kind="ExternalInput")
    t_diag = nc.dram_tensor("diag", [A, T, P, P], f32, kind="ExternalInput")
    t_sp = nc.dram_tensor("sp", [A, T, P, P], f32, kind="ExternalInput")
    t_qs = nc.dram_tensor("qs", [A, P, 1], f32, kind="ExternalInput")
    t_out = nc.dram_tensor("out", [A, R, D], f32, kind="ExternalOutput")

    Relu = mybir.ActivationFunctionType.Relu
    mult = mybir.AluOpType.mult
    add = mybir.AluOpType.add
    NCH = D // W
    NSUB = (W + BANK_F32 - 1) // BANK_F32
    WPAD = NSUB * BANK_F32

    with TileContext(nc) as tc:
        with (
            tc.tile_pool(name="wts", bufs=1) as wt_pool,
            tc.tile_pool(name="io", bufs=3) as io_pool,
            tc.tile_pool(name="io16", bufs=6) as io16_pool,
            tc.tile_pool(name="work", bufs=4) as work_pool,
            tc.tile_pool(name="psum", bufs=2, space="PSUM") as psum_pool,
        ):
            sp_sb, diag_sb, c1_sb = [], [], []
            for a in range(A):
                for t in range(T):
                    spt = wt_pool.tile([P, P], f32, name=f"sp_{a}_{t}")
                    nc.sync.dma_start(spt[:, :], t_sp[a, t])
                    dgt = wt_pool.tile([P, P], f32, name=f"diag_{a}_{t}")
                    nc.sync.dma_start(dgt[:, :], t_diag[a, t])
                    c1t = wt_pool.tile([P, 1], f32, name=f"c1_{a}_{t}")
                    nc.sync.dma_start(c1t[:, :], t_c1[a, t])
                    sp_sb.append(spt)
                    diag_sb.append(dgt)
                    c1_sb.append(c1t)
            qs_sb = []
            for a in range(A):
                qst = wt_pool.tile([P, 1], f32, name=f"qs_{a}")
                nc.sync.dma_start(qst[:, :], t_qs[a])
                qs_sb.append(qst)

            it = 0
            for t in range(T):
                rows = slice(t * P, (t + 1) * P)
                for ch in range(NCH):
                    cols = slice(ch * W, (ch + 1) * W)
                    xb_c = io_pool.tile([P, W], f32, name="xb_c", bufs=4)
                    nc.sync.dma_start(xb_c[:, :], t_xb[rows, cols])
                    xq_c = io_pool.tile([P, W], f32, name="xq_c", bufs=3)
                    nc.sync.dma_start(xq_c[:, :], t_xq[t, :, cols])
                    for a in range(A):
                        gn_c = io16_pool.tile([P, 3 * W], mybir.dt.uint8,
                                              name="gn_c")
                        nc.sync.dma_start(gn_c[:, :], t_gn[a, rows, ch])
                        gd_c = gn_c[:, 0:2 * W].bitcast(f16)
                        nz_c = gn_c[:, 2 * W:3 * W].bitcast(i8)

                        mix_on_pe = (it % 2 == 0)
                        it += 1
                        ps = psum_pool.tile([P, WPAD], f32, name="ps")
                        if mix_on_pe:
                            for s in range(NSUB):
                                lo = s * BANK_F32
                                hi = min(W, lo + BANK_F32)
                                nc.tensor.matmul(
                                    ps[:, lo:hi], diag_sb[a * T + t][:, :],
                                    xb_c[:, lo:hi], start=True, stop=False)
                            for s in range(NSUB):
                                lo = s * BANK_F32
                                hi = min(W, lo + BANK_F32)
                                nc.tensor.matmul(
                                    ps[:, lo:hi], sp_sb[a * T + t][:, :],
                                    xq_c[:, lo:hi], start=False, stop=True)
                        else:
                            for s in range(NSUB):
                                lo = s * BANK_F32
                                hi = min(W, lo + BANK_F32)
                                nc.tensor.matmul(
                                    ps[:, lo:hi], sp_sb[a * T + t][:, :],
                                    xq_c[:, lo:hi], start=True, stop=True)

                        sx = work_pool.tile([P, W], f32, name="sx")
                        if mix_on_pe:
                            # sx = psum * gd
                            nc.vector.tensor_tensor(sx[:, :], ps[:, 0:W],
                                                    gd_c, mult)
                        else:
                            # sx = c1*xb + psum, then *= gd
                            nc.vector.scalar_tensor_tensor(
                                sx[:, :], xb_c[:, :], c1_sb[a * T + t][:, :],
                                ps[:, 0:W], mult, add)
                            nc.vector.tensor_tensor(sx[:, :], sx[:, :],
                                                    gd_c, mult)
                        # sx += nz_i8 (dequant folded into gd and relu scale)
                        nc.gpsimd.tensor_tensor(sx[:, :], sx[:, :],
                                                nz_c, add)
                        out_c = io_pool.tile([P, W], f32, name="out_c", bufs=3)
                        nc.scalar.activation(out_c[:, :], sx[:, :], Relu,
                                             scale=qs_sb[a][:, :])
                        nc.scalar.dma_start(t_out[a, rows, cols], out_c[:, :])
    nc.finalize()
    return nc


def _get_program():
    key = (R, D, W, "v7")
    if key not in _PROGRAM_CACHE:
        _PROGRAM_CACHE[key] = _build_program()
    return _PROGRAM_CACHE[key]


# ----------------------------------------------------------------------------
# entry point
# ----------------------------------------------------------------------------
def kernel(x, ctx, y, cont_covs, cat_covs):
    from concourse.bass_utils import run_bass_kernel_spmd

    x = np.ascontiguousarray(np.asarray(x, dtype=np.float32))
    ctx = np.asarray(ctx)
    y = np.asarray(y)
    assert x.shape == (B, D), x.shape

    dec = _host_decompose(x, ctx, y)
    in_maps, overflow = _build_in_maps(x, dec)

    nc = _get_program()
    res = run_bass_kernel_spmd(nc, in_maps, core_ids=list(range(N_CORES)))

    counts = np.empty((N_AUG * B, D), np.float32)
    for c in range(N_CORES):
        out_c = res.results[c]["out"]
        for a in range(N_AUG):
            counts[a * B + c * R:(a * B + (c + 1) * R)] = out_c[a]
    _patch_overflow(counts, x, dec, overflow)

    aug_ctxs = dec["ctxs"].reshape(-1)
    y_rep = np.tile(y, N_AUG)
    cont_rep = np.tile(np.asarray(cont_covs), (N_AUG, 1)).reshape(-1)
    cat_rep = np.tile(np.asarray(cat_covs), (N_AUG, 1)).reshape(-1)
    return counts, aug_ctxs, y_rep, cont_rep, cat_rep


# revision 15
# speedup vs baseline: 1.8323x; 1.2078x over previous
"""Trainium2 Bass kernel for nn_BatchAugmentation_53850299957920.

Reference pipeline (3 augs of x[2048, 20000]):
  same-class/cross-dataset mixup -> dropout -> per-row scale -> per-gene scale
  -> + 0.01*gaussian noise -> relu; concat augs (plus repeated ctx/y/cov
  metadata outputs).

Work split:
  * Host (exact jax-CPU threefry — RNG must match the reference bit-exactly
    for every discrete decision): all random draws, the [B,B] mixup candidate
    search, per-row/per-gene scalars.  The O(B*D) random tensors ship
    compactly: keep*gscale as f16, noise as int8 + per-aug dequant scale.
  * Device (8 NeuronCores, data-parallel over batch rows): all heavy [B,D]
    elementwise math:

      psum = diag(c1) @ xb_tile + Sp @ xq_tile                          (PE)
      sx   = psum * gd'                                                 (DVE)
      sx  += nz_i8                                                   (GPSIMD)
      out  = relu((0.01*qs) * sx)                                       (ACT)

    c1 = 100*scale*lam_eff, Sp[k,p] = 100*scale*(1-lam) for parent slot k ->
    row p, gd' = keep*gscale/qs (f16), nz_i8 = noise quantized to int8 with
    per-aug scale qs.  Folding 1/qs into gd and qs into the relu's
    per-partition scale makes the noise add a plain integer tensor_tensor.
    xq holds up to 128 deduplicated parent rows per 128-row tile (shared
    across the 3 augs); rare overflow rows are patched on the host.  The
    mix/noise work alternates between engines so no single engine gates the
    DMA-bound pipeline.
"""
import sys

if "/opt/trn_rl_repo" not in sys.path:
    sys.path.insert(0, "/opt/trn_rl_repo")

import numpy as np

N_AUG = 3
DROPOUT = 0.2
DS_MIN = 0.8
DS_MAX = 1.2
MIXUP_P = 0.3
ALPHA = 0.4
GENE_P = 0.2

N_CORES = 8
P = 128
B, D = 2048, 20000
R = B // N_CORES          # rows per core
T = R // P                # row-tiles per core
W = 2000                  # column chunk
BANK_F32 = 512            # one PSUM bank in f32 elements

_PROGRAM_CACHE = {}


# ----------------------------------------------------------------------------
# host side: exact RNG decomposition (jax CPU threefry == reference bits)
# ----------------------------------------------------------------------------
def _host_decompose(x, ctx, y):
    import jax
    import jax.numpy as jnp

    cpu = jax.devices("cpu")[0]
    Bx, Dx = x.shape
    base_key = jax.device_put(jax.random.key(42), cpu)

    c1 = np.empty((N_AUG, Bx), np.float32)
    c2 = np.empty((N_AUG, Bx), np.float32)
    j_sel_all = np.empty((N_AUG, Bx), np.int64)
    do_mix_all = np.empty((N_AUG, Bx), bool)
    ctxs = []
    keep_list = []
    gscale_list = []
    noise_list = []
    qs = np.empty((N_AUG,), np.float32)

    yy = np.asarray(y)
    cc = np.asarray(ctx)

    with jax.default_device(cpu):
        for a in range(N_AUG):
            k = jax.random.fold_in(base_key, a)
            k_mix, k_drop, k_scale, k_gmask, k_gscale, k_noise = \
                jax.random.split(k, 6)

            # --- mixup selection (bit-exact vs reference) ---
            k_sel, k_pick, k_lam, k_coin = jax.random.split(k_mix, 4)
            mix_mask = np.asarray(jax.random.uniform(k_sel, (Bx,))) < MIXUP_P
            cand = (yy[None, :] == yy[:, None]) & (cc[None, :] != cc[:, None])
            counts = cand.sum(axis=1).astype(np.int32)
            u = np.asarray(jax.random.uniform(k_pick, (Bx,)))
            kth = np.minimum(
                np.floor(u * counts.astype(np.float32)).astype(np.int32),
                np.maximum(counts - 1, 0),
            )
            csum = np.cumsum(cand.astype(np.int32), axis=1)
            j_sel = np.argmax(csum > kth[:, None], axis=1)
            lam = np.asarray(
                jax.random.beta(k_lam, ALPHA, ALPHA, (Bx,), dtype=jnp.float32)
            )
            do_mix = mix_mask & (counts > 0)
            coin = np.asarray(jax.random.uniform(k_coin, (Bx,))) < 0.5
            ctx_new = np.where(do_mix & (~coin), cc[j_sel], cc).astype(cc.dtype)

            scale = (
                np.asarray(jax.random.uniform(k_scale, (Bx, 1)))[:, 0]
                * (DS_MAX - DS_MIN) + DS_MIN
            ).astype(np.float32)
            lam_eff = np.where(do_mix, lam, np.float32(1.0)).astype(np.float32)
            c1[a] = 100.0 * scale * lam_eff
            c2[a] = np.where(do_mix, 100.0 * scale * (1.0 - lam_eff), 0.0)
            j_sel_all[a] = j_sel
            do_mix_all[a] = do_mix
            ctxs.append(ctx_new)

            gmask = np.asarray(jax.random.uniform(k_gmask, (Dx,))) < GENE_P
            gs_draw = np.asarray(jax.random.uniform(k_gscale, (Dx,))) * 0.4 + 0.8
            gscale_list.append(np.where(gmask, gs_draw, 1.0).astype(np.float32))

            # big exact draws
            keep_list.append(
                np.asarray(jax.random.uniform(k_drop, (Bx, Dx))) > DROPOUT
            )
            n = np.asarray(
                jax.random.normal(k_noise, (Bx, Dx), dtype=jnp.float32)
            )
            qs[a] = np.float32(np.abs(n).max() / 127.0)
            noise_list.append(
                np.clip(np.rint(n / qs[a]), -127, 127).astype(np.int8)
            )

    return dict(c1=c1, c2=c2, j_sel=j_sel_all, do_mix=do_mix_all,
                ctxs=np.stack(ctxs, axis=0), keep=keep_list,
                gscale=gscale_list, noise_i8=noise_list, qs=qs)


# ----------------------------------------------------------------------------
# per-core input assembly
# ----------------------------------------------------------------------------
def _build_in_maps(x, dec):
    """Returns (in_maps, overflow) where overflow is a list of
    (a, global_row) whose parent didn't fit in the 128-slot cap."""
    qs_full = np.ascontiguousarray(
        np.broadcast_to((np.float32(0.01) * dec["qs"])[:, None, None],
                        (N_AUG, P, 1)).astype(np.float32))
    in_maps = []
    overflow = []
    NCH = D // W
    for c in range(N_CORES):
        rows = slice(c * R, (c + 1) * R)
        # packed per-chunk stream, all augs: [gd f16 bytes | nz i8 bytes] x A
        gn = np.empty((R, NCH, N_AUG * 3 * W), np.uint8)
        gnv = gn.reshape(R, NCH, N_AUG, 3 * W)
        for a in range(N_AUG):
            gd_a = (dec["keep"][a][rows] *
                    (dec["gscale"][a] / dec["qs"][a])[None, :]).astype(
                        np.float16)
            gnv[:, :, a, :2 * W] = \
                gd_a.reshape(R, NCH, W).view(np.uint8).transpose(0, 1, 2)
            gnv[:, :, a, 2 * W:] = \
                dec["noise_i8"][a][rows].reshape(R, NCH, W).view(np.uint8)

        xq = np.zeros((T, P, D), np.float32)
        diag = np.zeros((N_AUG, T, P, P), np.float32)
        sp = np.zeros((N_AUG, T, P, P), np.float32)
        ar = np.arange(P)
        for t in range(T):
            slot_of = {}          # parent row j -> slot
            slot_js = []
            for a in range(N_AUG):
                base = c * R + t * P
                diag[a, t, ar, ar] = dec["c1"][a, base:base + P]
                mix_p = np.nonzero(dec["do_mix"][a, base:base + P])[0]
                for p in mix_p:
                    j = int(dec["j_sel"][a, base + p])
                    k = slot_of.get(j)
                    if k is None:
                        if len(slot_js) >= P:
                            overflow.append((a, base + p))
                            continue
                        k = len(slot_js)
                        slot_of[j] = k
                        slot_js.append(j)
                    sp[a, t, k, p] = dec["c2"][a, base + p]
            if slot_js:
                xq[t, :len(slot_js)] = x[np.asarray(slot_js)]

        in_maps.append({
            "xb": x[rows], "xq": xq, "gn": gn,
            "c1": np.ascontiguousarray(
                dec["c1"][:, rows].reshape(N_AUG, T, P, 1)),
            "diag": diag, "sp": sp, "qs": qs_full,
        })
    return in_maps, overflow


def _patch_overflow(counts, x, dec, overflow):
    """Host-exact recompute of rows whose mixup parent had no xq slot."""
    for a, i in overflow:
        gd_row = (dec["keep"][a][i] * dec["gscale"][a]).astype(np.float16)
        s = dec["c1"][a, i] * x[i] + dec["c2"][a, i] * x[dec["j_sel"][a, i]]
        h = gd_row.astype(np.float32) * s + \
            dec["qs"][a] * dec["noise_i8"][a][i].astype(np.float32)
        counts[a * B + i] = np.maximum(np.float32(0.01) * h, 0.0)


# ----------------------------------------------------------------------------
# device program (v3)
# ----------------------------------------------------------------------------
def _build_program():
    import concourse.bacc as bacc
    from concourse import mybir
    from concourse.tile import TileContext

    A = N_AUG
    nc = bacc.Bacc("TRN2")
    f32, f16, i8 = mybir.dt.float32, mybir.dt.float16, mybir.dt.int8
    t_xb = nc.dram_tensor("xb", [R, D], f32, kind="ExternalInput")
    t_xq = nc.dram_tensor("xq", [T, P, D], f32, kind="ExternalInput")
    NCHd = D // W
    t_gn = nc.dram_tensor("gn", [R, NCHd, A * 3 * W], mybir.dt.uint8,
                          kind="ExternalInput")
    t_c1 = nc.dram_tensor("c1", [A, T, P, 1], f32, kind="ExternalInput")
    t_diag = nc.dram_tensor("diag", [A, T, P, P], f32, kind="ExternalInput")
    t_sp = nc.dram_tensor("sp", [A, T, P, P], f32, kind="ExternalInput")
    t_qs = nc.dram_tensor("qs", [A, P, 1], f32, kind="ExternalInput")
    t_out = nc.dram_tensor("out", [A, R, D], f16, kind="ExternalOutput")

    Relu = mybir.ActivationFunctionType.Relu
    mult = mybir.AluOpType.mult
    add = mybir.AluOpType.add
    NCH = D // W
    NSUB = (W + BANK_F32 - 1) // BANK_F32
    WPAD = NSUB * BANK_F32

    with TileContext(nc) as tc:
        with (
            tc.tile_pool(name="wts", bufs=1) as wt_pool,
            tc.tile_pool(name="io", bufs=3) as io_pool,
            tc.tile_pool(name="io16", bufs=6) as io16_pool,
            tc.tile_pool(name="work", bufs=4) as work_pool,
            tc.tile_pool(name="psum", bufs=2, space="PSUM") as psum_pool,
        ):
            sp_sb, diag_sb, c1_sb = [], [], []
            for a in range(A):
                for t in range(T):
                    spt = wt_pool.tile([P, P], f32, name=f"sp_{a}_{t}")
                    nc.sync.dma_start(spt[:, :], t_sp[a, t])
                    dgt = wt_pool.tile([P, P], f32, name=f"diag_{a}_{t}")
                    nc.sync.dma_start(dgt[:, :], t_diag[a, t])
                    c1t = wt_pool.tile([P, 1], f32, name=f"c1_{a}_{t}")
                    nc.sync.dma_start(c1t[:, :], t_c1[a, t])
                    sp_sb.append(spt)
                    diag_sb.append(dgt)
                    c1_sb.append(c1t)
            qs_sb = []
            for a in range(A):
                qst = wt_pool.tile([P, 1], f32, name=f"qs_{a}")
                nc.sync.dma_start(qst[:, :], t_qs[a])
                qs_sb.append(qst)

            it = 0
            for t in range(T):
                rows = slice(t * P, (t + 1) * P)
                for ch in range(NCH):
                    cols = slice(ch * W, (ch + 1) * W)
                    xb_c = io_pool.tile([P, W], f32, name="xb_c", bufs=4)
                    nc.sync.dma_start(xb_c[:, :], t_xb[rows, cols])
                    xq_c = io_pool.tile([P, W], f32, name="xq_c", bufs=3)
                    nc.sync.dma_start(xq_c[:, :], t_xq[t, :, cols])
                    gn_c = io16_pool.tile([P, A * 3 * W], mybir.dt.uint8,
                                          name="gn_c", bufs=3)
                    nc.sync.dma_start(gn_c[:, :], t_gn[rows, ch])
                    for a in range(A):
                        gd_c = gn_c[:, a * 3 * W:a * 3 * W + 2 * W].bitcast(f16)
                        nz_c = gn_c[:, a * 3 * W + 2 * W:(a + 1) * 3 * W].bitcast(i8)

                        mix_on_pe = (it % 2 == 0)
                        it += 1
                        ps = psum_pool.tile([P, WPAD], f32, name="ps")
                        if mix_on_pe:
                            for s in range(NSUB):
                                lo = s * BANK_F32
                                hi = min(W, lo + BANK_F32)
                                nc.tensor.matmul(
                                    ps[:, lo:hi], diag_sb[a * T + t][:, :],
                                    xb_c[:, lo:hi], start=True, stop=False)
                            for s in range(NSUB):
                                lo = s * BANK_F32
                                hi = min(W, lo + BANK_F32)
                                nc.tensor.matmul(
                                    ps[:, lo:hi], sp_sb[a * T + t][:, :],
                                    xq_c[:, lo:hi], start=False, stop=True)
                        else:
                            for s in range(NSUB):
                                lo = s * BANK_F32
                                hi = min(W, lo + BANK_F32)
                                nc.tensor.matmul(
                                    ps[:, lo:hi], sp_sb[a * T + t][:, :],
                                    xq_c[:, lo:hi], start=True, stop=True)

                        sx = work_pool.tile([P, W], f32, name="sx")
                        if mix_on_pe:
                            # sx = psum * gd
                            nc.vector.tensor_tensor(sx[:, :], ps[:, 0:W],
                                                    gd_c, mult)
                        else:
                            # sx = c1*xb + psum, then *= gd
                            nc.vector.scalar_tensor_tensor(
                                sx[:, :], xb_c[:, :], c1_sb[a * T + t][:, :],
                                ps[:, 0:W], mult, add)
                            nc.vector.tensor_tensor(sx[:, :], sx[:, :],
                                                    gd_c, mult)
                        # sx += nz_i8 (dequant folded into gd and relu scale)
                        nc.gpsimd.tensor_tensor(sx[:, :], sx[:, :],
                                                nz_c, add)
                        out_c = io_pool.tile([P, W], f16, name="out_c", bufs=4)
                        nc.scalar.activation(out_c[:, :], sx[:, :], Relu,
                                             scale=qs_sb[a][:, :])
                        nc.scalar.dma_start(t_out[a, rows, cols], out_c[:, :])
    nc.finalize()
    return nc


def _get_program():
    key = (R, D, W, "v8")
    if key not in _PROGRAM_CACHE:
        _PROGRAM_CACHE[key] = _build_program()
    return _PROGRAM_CACHE[key]


# ----------------------------------------------------------------------------
# entry point
# ----------------------------------------------------------------------------
def kernel(x, ctx, y, cont_covs, cat_covs):
    from concourse.bass_utils import run_bass_kernel_spmd

    x = np.ascontiguousarray(np.asarray(x, dtype=np.float32))
    ctx = np.asarray(ctx)
    y = np.asarray(y)
    assert x.shape == (B, D), x.shape

    dec = _host_decompose(x, ctx, y)
    in_maps, overflow = _build_in_maps(x, dec)

    nc = _get_program()
    res = run_bass_kernel_spmd(nc, in_maps, core_ids=list(range(N_CORES)))

    counts = np.empty((N_AUG * B, D), np.float32)
    for c in range(N_CORES):
        out_c = res.results[c]["out"]
        for a in range(N_AUG):
            counts[a * B + c * R:(a * B + (c + 1) * R)] = \
                out_c[a].astype(np.float32)
    _patch_overflow(counts, x, dec, overflow)

    aug_ctxs = dec["ctxs"].reshape(-1)
    y_rep = np.tile(y, N_AUG)
    cont_rep = np.tile(np.asarray(cont_covs), (N_AUG, 1)).reshape(-1)
    cat_rep = np.tile(np.asarray(cat_covs), (N_AUG, 1)).reshape(-1)
    return counts, aug_ctxs, y_rep, cont_rep, cat_rep


# revision 16
# speedup vs baseline: 1.8562x; 1.0131x over previous
"""Trainium2 Bass kernel for nn_BatchAugmentation_53850299957920.

Reference pipeline (3 augs of x[2048, 20000]):
  same-class/cross-dataset mixup -> dropout -> per-row scale -> per-gene scale
  -> + 0.01*gaussian noise -> relu; concat augs (plus repeated ctx/y/cov
  metadata outputs).

Work split:
  * Host (exact jax-CPU threefry — RNG must match the reference bit-exactly
    for every discrete decision): all random draws, the [B,B] mixup candidate
    search, per-row/per-gene scalars.  The O(B*D) random tensors ship
    compactly: keep*gscale as f16, noise as int8 + per-aug dequant scale.
  * Device (8 NeuronCores, data-parallel over batch rows): all heavy [B,D]
    elementwise math:

      psum = diag(c1) @ xb_tile + Sp @ xq_tile                          (PE)
      sx   = psum * gd'                                                 (DVE)
      sx  += nz_i8                                                   (GPSIMD)
      out  = relu((0.01*qs) * sx)                                       (ACT)

    c1 = 100*scale*lam_eff, Sp[k,p] = 100*scale*(1-lam) for parent slot k ->
    row p, gd' = keep*gscale/qs (f16), nz_i8 = noise quantized to int8 with
    per-aug scale qs.  Folding 1/qs into gd and qs into the relu's
    per-partition scale makes the noise add a plain integer tensor_tensor.
    xq holds up to 128 deduplicated parent rows per 128-row tile (shared
    across the 3 augs); rare overflow rows are patched on the host.  The
    mix/noise work alternates between engines so no single engine gates the
    DMA-bound pipeline.
"""
import sys

if "/opt/trn_rl_repo" not in sys.path:
    sys.path.insert(0, "/opt/trn_rl_repo")

import numpy as np

N_AUG = 3
DROPOUT = 0.2
DS_MIN = 0.8
DS_MAX = 1.2
MIXUP_P = 0.3
ALPHA = 0.4
GENE_P = 0.2

N_CORES = 8
P = 128
B, D = 2048, 20000
R = B // N_CORES          # rows per core
T = R // P                # row-tiles per core
W = 2000                  # column chunk
BANK_F32 = 512            # one PSUM bank in f32 elements

_PROGRAM_CACHE = {}


# ----------------------------------------------------------------------------
# host side: exact RNG decomposition (jax CPU threefry == reference bits)
# ----------------------------------------------------------------------------
def _host_decompose(x, ctx, y):
    import jax
    import jax.numpy as jnp

    cpu = jax.devices("cpu")[0]
    Bx, Dx = x.shape
    base_key = jax.device_put(jax.random.key(42), cpu)

    c1 = np.empty((N_AUG, Bx), np.float32)
    c2 = np.empty((N_AUG, Bx), np.float32)
    j_sel_all = np.empty((N_AUG, Bx), np.int64)
    do_mix_all = np.empty((N_AUG, Bx), bool)
    ctxs = []
    keep_list = []
    gscale_list = []
    noise_list = []
    qs = np.empty((N_AUG,), np.float32)

    yy = np.asarray(y)
    cc = np.asarray(ctx)

    with jax.default_device(cpu):
        for a in range(N_AUG):
            k = jax.random.fold_in(base_key, a)
            k_mix, k_drop, k_scale, k_gmask, k_gscale, k_noise = \
                jax.random.split(k, 6)

            # --- mixup selection (bit-exact vs reference) ---
            k_sel, k_pick, k_lam, k_coin = jax.random.split(k_mix, 4)
            mix_mask = np.asarray(jax.random.uniform(k_sel, (Bx,))) < MIXUP_P
            cand = (yy[None, :] == yy[:, None]) & (cc[None, :] != cc[:, None])
            counts = cand.sum(axis=1).astype(np.int32)
            u = np.asarray(jax.random.uniform(k_pick, (Bx,)))
            kth = np.minimum(
                np.floor(u * counts.astype(np.float32)).astype(np.int32),
                np.maximum(counts - 1, 0),
            )
            csum = np.cumsum(cand.astype(np.int32), axis=1)
            j_sel = np.argmax(csum > kth[:, None], axis=1)
            lam = np.asarray(
                jax.random.beta(k_lam, ALPHA, ALPHA, (Bx,), dtype=jnp.float32)
            )
            do_mix = mix_mask & (counts > 0)
            coin = np.asarray(jax.random.uniform(k_coin, (Bx,))) < 0.5
            ctx_new = np.where(do_mix & (~coin), cc[j_sel], cc).astype(cc.dtype)

            scale = (
                np.asarray(jax.random.uniform(k_scale, (Bx, 1)))[:, 0]
                * (DS_MAX - DS_MIN) + DS_MIN
            ).astype(np.float32)
            lam_eff = np.where(do_mix, lam, np.float32(1.0)).astype(np.float32)
            c1[a] = 100.0 * scale * lam_eff
            c2[a] = np.where(do_mix, 100.0 * scale * (1.0 - lam_eff), 0.0)
            j_sel_all[a] = j_sel
            do_mix_all[a] = do_mix
            ctxs.append(ctx_new)

            gmask = np.asarray(jax.random.uniform(k_gmask, (Dx,))) < GENE_P
            gs_draw = np.asarray(jax.random.uniform(k_gscale, (Dx,))) * 0.4 + 0.8
            gscale_list.append(np.where(gmask, gs_draw, 1.0).astype(np.float32))

            # big exact draws
            keep_list.append(
                np.asarray(jax.random.uniform(k_drop, (Bx, Dx))) > DROPOUT
            )
            n = np.asarray(
                jax.random.normal(k_noise, (Bx, Dx), dtype=jnp.float32)
            )
            qs[a] = np.float32(np.abs(n).max() / 127.0)
            noise_list.append(
                np.clip(np.rint(n / qs[a]), -127, 127).astype(np.int8)
            )

    return dict(c1=c1, c2=c2, j_sel=j_sel_all, do_mix=do_mix_all,
                ctxs=np.stack(ctxs, axis=0), keep=keep_list,
                gscale=gscale_list, noise_i8=noise_list, qs=qs)


# ----------------------------------------------------------------------------
# per-core input assembly
# ----------------------------------------------------------------------------
def _build_in_maps(x, dec):
    """Returns (in_maps, overflow) where overflow is a list of
    (a, global_row) whose parent didn't fit in the 128-slot cap."""
    qs_full = np.ascontiguousarray(
        np.broadcast_to((np.float32(0.01) * dec["qs"])[:, None, None],
                        (N_AUG, P, 1)).astype(np.float32))
    in_maps = []
    overflow = []
    NCH = D // W
    for c in range(N_CORES):
        rows = slice(c * R, (c + 1) * R)
        # packed per-chunk stream, all augs: [gd f16 bytes | nz i8 bytes] x A
        gn = np.empty((R, NCH, N_AUG * 3 * W), np.uint8)
        gnv = gn.reshape(R, NCH, N_AUG, 3 * W)
        for a in range(N_AUG):
            gd_a = (dec["keep"][a][rows] *
                    (dec["gscale"][a] / dec["qs"][a])[None, :]).astype(
                        np.float16)
            gnv[:, :, a, :2 * W] = \
                gd_a.reshape(R, NCH, W).view(np.uint8).transpose(0, 1, 2)
            gnv[:, :, a, 2 * W:] = \
                dec["noise_i8"][a][rows].reshape(R, NCH, W).view(np.uint8)

        xq = np.zeros((T, P, D), np.float32)
        diag = np.zeros((N_AUG, T, P, P), np.float32)
        sp = np.zeros((N_AUG, T, P, P), np.float32)
        ar = np.arange(P)
        for t in range(T):
            slot_of = {}          # parent row j -> slot
            slot_js = []
            for a in range(N_AUG):
                base = c * R + t * P
                diag[a, t, ar, ar] = dec["c1"][a, base:base + P]
                mix_p = np.nonzero(dec["do_mix"][a, base:base + P])[0]
                for p in mix_p:
                    j = int(dec["j_sel"][a, base + p])
                    k = slot_of.get(j)
                    if k is None:
                        if len(slot_js) >= P:
                            overflow.append((a, base + p))
                            continue
                        k = len(slot_js)
                        slot_of[j] = k
                        slot_js.append(j)
                    sp[a, t, k, p] = dec["c2"][a, base + p]
            if slot_js:
                xq[t, :len(slot_js)] = x[np.asarray(slot_js)]

        in_maps.append({
            "xb": x[rows], "xq": xq, "gn": gn,
            "c1": np.ascontiguousarray(
                dec["c1"][:, rows].reshape(N_AUG, T, P, 1)),
            "diag": diag, "sp": sp, "qs": qs_full,
        })
    return in_maps, overflow


def _patch_overflow(counts, x, dec, overflow):
    """Host-exact recompute of rows whose mixup parent had no xq slot."""
    for a, i in overflow:
        gd_row = (dec["keep"][a][i] * dec["gscale"][a]).astype(np.float16)
        s = dec["c1"][a, i] * x[i] + dec["c2"][a, i] * x[dec["j_sel"][a, i]]
        h = gd_row.astype(np.float32) * s + \
            dec["qs"][a] * dec["noise_i8"][a][i].astype(np.float32)
        counts[a * B + i] = np.maximum(np.float32(0.01) * h, 0.0)


# ----------------------------------------------------------------------------
# device program (v3)
# ----------------------------------------------------------------------------
def _build_program():
    import concourse.bacc as bacc
    from concourse import mybir
    from concourse.tile import TileContext

    A = N_AUG
    nc = bacc.Bacc("TRN2")
    f32, f16, i8 = mybir.dt.float32, mybir.dt.float16, mybir.dt.int8
    t_xb = nc.dram_tensor("xb", [R, D], f32, kind="ExternalInput")
    t_xq = nc.dram_tensor("xq", [T, P, D], f32, kind="ExternalInput")
    NCHd = D // W
    t_gn = nc.dram_tensor("gn", [R, NCHd, A * 3 * W], mybir.dt.uint8,
                          kind="ExternalInput")
    t_c1 = nc.dram_tensor("c1", [A, T, P, 1], f32, kind="ExternalInput")
    t_diag = nc.dram_tensor("diag", [A, T, P, P], f32, kind="ExternalInput")
    t_sp = nc.dram_tensor("sp", [A, T, P, P], f32, kind="ExternalInput")
    t_qs = nc.dram_tensor("qs", [A, P, 1], f32, kind="ExternalInput")
    t_out = nc.dram_tensor("out", [A, R, D], f16, kind="ExternalOutput")

    Relu = mybir.ActivationFunctionType.Relu
    mult = mybir.AluOpType.mult
    add = mybir.AluOpType.add
    NCH = D // W
    NSUB = (W + BANK_F32 - 1) // BANK_F32
    WPAD = NSUB * BANK_F32

    with TileContext(nc) as tc:
        with (
            tc.tile_pool(name="wts", bufs=1) as wt_pool,
            tc.tile_pool(name="io", bufs=3) as io_pool,
            tc.tile_pool(name="io16", bufs=6) as io16_pool,
            tc.tile_pool(name="work", bufs=4) as work_pool,
            tc.tile_pool(name="psum", bufs=2, space="PSUM") as psum_pool,
        ):
            sp_sb, diag_sb, c1_sb = [], [], []
            for a in range(A):
                for t in range(T):
                    spt = wt_pool.tile([P, P], f32, name=f"sp_{a}_{t}")
                    nc.sync.dma_start(spt[:, :], t_sp[a, t])
                    dgt = wt_pool.tile([P, P], f32, name=f"diag_{a}_{t}")
                    nc.sync.dma_start(dgt[:, :], t_diag[a, t])
                    c1t = wt_pool.tile([P, 1], f32, name=f"c1_{a}_{t}")
                    nc.sync.dma_start(c1t[:, :], t_c1[a, t])
                    sp_sb.append(spt)
                    diag_sb.append(dgt)
                    c1_sb.append(c1t)
            qs_sb = []
            for a in range(A):
                qst = wt_pool.tile([P, 1], f32, name=f"qs_{a}")
                nc.sync.dma_start(qst[:, :], t_qs[a])
                qs_sb.append(qst)

            it = 0
            for t in range(T):
                rows = slice(t * P, (t + 1) * P)
                for ch in range(NCH):
                    cols = slice(ch * W, (ch + 1) * W)
                    xb_c = io_pool.tile([P, W], f32, name="xb_c", bufs=4)
                    nc.sync.dma_start(xb_c[:, :], t_xb[rows, cols])
                    xq_c = io_pool.tile([P, W], f32, name="xq_c", bufs=3)
                    nc.sync.dma_start(xq_c[:, :], t_xq[t, :, cols])
                    gn_c = io16_pool.tile([P, A * 3 * W], mybir.dt.uint8,
                                          name="gn_c", bufs=3)
                    nc.sync.dma_start(gn_c[:, :], t_gn[rows, ch])
                    for a in range(A):
                        gd_c = gn_c[:, a * 3 * W:a * 3 * W + 2 * W].bitcast(f16)
                        nz_c = gn_c[:, a * 3 * W + 2 * W:(a + 1) * 3 * W].bitcast(i8)

                        mix_on_pe = (it % 4 == 0)
                        it += 1
                        ps = psum_pool.tile([P, WPAD], f32, name="ps")
                        if mix_on_pe:
                            for s in range(NSUB):
                                lo = s * BANK_F32
                                hi = min(W, lo + BANK_F32)
                                nc.tensor.matmul(
                                    ps[:, lo:hi], diag_sb[a * T + t][:, :],
                                    xb_c[:, lo:hi], start=True, stop=False)
                            for s in range(NSUB):
                                lo = s * BANK_F32
                                hi = min(W, lo + BANK_F32)
                                nc.tensor.matmul(
                                    ps[:, lo:hi], sp_sb[a * T + t][:, :],
                                    xq_c[:, lo:hi], start=False, stop=True)
                        else:
                            for s in range(NSUB):
                                lo = s * BANK_F32
                                hi = min(W, lo + BANK_F32)
                                nc.tensor.matmul(
                                    ps[:, lo:hi], sp_sb[a * T + t][:, :],
                                    xq_c[:, lo:hi], start=True, stop=True)

                        sx = work_pool.tile([P, W], f32, name="sx")
                        if mix_on_pe:
                            # sx = psum * gd
                            nc.vector.tensor_tensor(sx[:, :], ps[:, 0:W],
                                                    gd_c, mult)
                        else:
                            # sx = c1*xb + psum, then *= gd
                            nc.vector.scalar_tensor_tensor(
                                sx[:, :], xb_c[:, :], c1_sb[a * T + t][:, :],
                                ps[:, 0:W], mult, add)
                            nc.vector.tensor_tensor(sx[:, :], sx[:, :],
                                                    gd_c, mult)
                        # sx += nz_i8 (dequant folded into gd and relu scale)
                        nc.gpsimd.tensor_tensor(sx[:, :], sx[:, :],
                                                nz_c, add)
                        out_c = io_pool.tile([P, W], f16, name="out_c", bufs=4)
                        nc.scalar.activation(out_c[:, :], sx[:, :], Relu,
                                             scale=qs_sb[a][:, :])
                        nc.scalar.dma_start(t_out[a, rows, cols], out_c[:, :])
    nc.finalize()
    return nc


def _get_program():
    key = (R, D, W, "v9")
    if key not in _PROGRAM_CACHE:
        _PROGRAM_CACHE[key] = _build_program()
    return _PROGRAM_CACHE[key]


# ----------------------------------------------------------------------------
# entry point
# ----------------------------------------------------------------------------
def kernel(x, ctx, y, cont_covs, cat_covs):
    from concourse.bass_utils import run_bass_kernel_spmd

    x = np.ascontiguousarray(np.asarray(x, dtype=np.float32))
    ctx = np.asarray(ctx)
    y = np.asarray(y)
    assert x.shape == (B, D), x.shape

    dec = _host_decompose(x, ctx, y)
    in_maps, overflow = _build_in_maps(x, dec)

    nc = _get_program()
    res = run_bass_kernel_spmd(nc, in_maps, core_ids=list(range(N_CORES)))

    counts = np.empty((N_AUG * B, D), np.float32)
    for c in range(N_CORES):
        out_c = res.results[c]["out"]
        for a in range(N_AUG):
            counts[a * B + c * R:(a * B + (c + 1) * R)] = \
                out_c[a].astype(np.float32)
    _patch_overflow(counts, x, dec, overflow)

    aug_ctxs = dec["ctxs"].reshape(-1)
    y_rep = np.tile(y, N_AUG)
    cont_rep = np.tile(np.asarray(cont_covs), (N_AUG, 1)).reshape(-1)
    cat_rep = np.tile(np.asarray(cat_covs), (N_AUG, 1)).reshape(-1)
    return counts, aug_ctxs, y_rep, cont_rep, cat_rep


# revision 17
# speedup vs baseline: 1.9045x; 1.0260x over previous
"""Trainium2 Bass kernel for nn_BatchAugmentation_53850299957920.

Reference pipeline (3 augs of x[2048, 20000]):
  same-class/cross-dataset mixup -> dropout -> per-row scale -> per-gene scale
  -> + 0.01*gaussian noise -> relu; concat augs (plus repeated ctx/y/cov
  metadata outputs).

Work split:
  * Host (exact jax-CPU threefry — RNG must match the reference bit-exactly
    for every discrete decision): all random draws, the [B,B] mixup candidate
    search, per-row/per-gene scalars.  The O(B*D) random tensors ship
    compactly: keep*gscale as f16, noise as int8 + per-aug dequant scale.
  * Device (8 NeuronCores, data-parallel over batch rows): all heavy [B,D]
    elementwise math:

      psum = diag(c1) @ xb_tile + Sp @ xq_tile                          (PE)
      sx   = psum * gd'                                                 (DVE)
      sx  += nz_i8                                                   (GPSIMD)
      out  = relu((0.01*qs) * sx)                                       (ACT)

    c1 = 100*scale*lam_eff, Sp[k,p] = 100*scale*(1-lam) for parent slot k ->
    row p, gd' = keep*gscale/qs (f16), nz_i8 = noise quantized to int8 with
    per-aug scale qs.  Folding 1/qs into gd and qs into the relu's
    per-partition scale makes the noise add a plain integer tensor_tensor.
    xq holds up to 128 deduplicated parent rows per 128-row tile (shared
    across the 3 augs); rare overflow rows are patched on the host.  The
    mix/noise work alternates between engines so no single engine gates the
    DMA-bound pipeline.
"""
import sys

if "/opt/trn_rl_repo" not in sys.path:
    sys.path.insert(0, "/opt/trn_rl_repo")

import numpy as np

N_AUG = 3
DROPOUT = 0.2
DS_MIN = 0.8
DS_MAX = 1.2
MIXUP_P = 0.3
ALPHA = 0.4
GENE_P = 0.2

N_CORES = 8
P = 128
B, D = 2048, 20000
R = B // N_CORES          # rows per core
T = R // P                # row-tiles per core
W = 2000                  # column chunk
BANK_F32 = 512            # one PSUM bank in f32 elements

_PROGRAM_CACHE = {}


# ----------------------------------------------------------------------------
# host side: exact RNG decomposition (jax CPU threefry == reference bits)
# ----------------------------------------------------------------------------
def _host_decompose(x, ctx, y):
    import jax
    import jax.numpy as jnp

    cpu = jax.devices("cpu")[0]
    Bx, Dx = x.shape
    base_key = jax.device_put(jax.random.key(42), cpu)

    c1 = np.empty((N_AUG, Bx), np.float32)
    c2 = np.empty((N_AUG, Bx), np.float32)
    j_sel_all = np.empty((N_AUG, Bx), np.int64)
    do_mix_all = np.empty((N_AUG, Bx), bool)
    ctxs = []
    keep_list = []
    gscale_list = []
    noise_list = []
    qs = np.empty((N_AUG,), np.float32)

    yy = np.asarray(y)
    cc = np.asarray(ctx)

    with jax.default_device(cpu):
        for a in range(N_AUG):
            k = jax.random.fold_in(base_key, a)
            k_mix, k_drop, k_scale, k_gmask, k_gscale, k_noise = \
                jax.random.split(k, 6)

            # --- mixup selection (bit-exact vs reference) ---
            k_sel, k_pick, k_lam, k_coin = jax.random.split(k_mix, 4)
            mix_mask = np.asarray(jax.random.uniform(k_sel, (Bx,))) < MIXUP_P
            cand = (yy[None, :] == yy[:, None]) & (cc[None, :] != cc[:, None])
            counts = cand.sum(axis=1).astype(np.int32)
            u = np.asarray(jax.random.uniform(k_pick, (Bx,)))
            kth = np.minimum(
                np.floor(u * counts.astype(np.float32)).astype(np.int32),
                np.maximum(counts - 1, 0),
            )
            csum = np.cumsum(cand.astype(np.int32), axis=1)
            j_sel = np.argmax(csum > kth[:, None], axis=1)
            lam = np.asarray(
                jax.random.beta(k_lam, ALPHA, ALPHA, (Bx,), dtype=jnp.float32)
            )
            do_mix = mix_mask & (counts > 0)
            coin = np.asarray(jax.random.uniform(k_coin, (Bx,))) < 0.5
            ctx_new = np.where(do_mix & (~coin), cc[j_sel], cc).astype(cc.dtype)

            scale = (
                np.asarray(jax.random.uniform(k_scale, (Bx, 1)))[:, 0]
                * (DS_MAX - DS_MIN) + DS_MIN
            ).astype(np.float32)
            lam_eff = np.where(do_mix, lam, np.float32(1.0)).astype(np.float32)
            c1[a] = 100.0 * scale * lam_eff
            c2[a] = np.where(do_mix, 100.0 * scale * (1.0 - lam_eff), 0.0)
            j_sel_all[a] = j_sel
            do_mix_all[a] = do_mix
            ctxs.append(ctx_new)

            gmask = np.asarray(jax.random.uniform(k_gmask, (Dx,))) < GENE_P
            gs_draw = np.asarray(jax.random.uniform(k_gscale, (Dx,))) * 0.4 + 0.8
            gscale_list.append(np.where(gmask, gs_draw, 1.0).astype(np.float32))

            # big exact draws
            keep_list.append(
                np.asarray(jax.random.uniform(k_drop, (Bx, Dx))) > DROPOUT
            )
            n = np.asarray(
                jax.random.normal(k_noise, (Bx, Dx), dtype=jnp.float32)
            )
            qs[a] = np.float32(np.abs(n).max() / 127.0)
            noise_list.append(
                np.clip(np.rint(n / qs[a]), -127, 127).astype(np.int8)
            )

    return dict(c1=c1, c2=c2, j_sel=j_sel_all, do_mix=do_mix_all,
                ctxs=np.stack(ctxs, axis=0), keep=keep_list,
                gscale=gscale_list, noise_i8=noise_list, qs=qs)


# ----------------------------------------------------------------------------
# per-core input assembly
# ----------------------------------------------------------------------------
def _build_in_maps(x, dec):
    """Returns (in_maps, overflow) where overflow is a list of
    (a, global_row) whose parent didn't fit in the 128-slot cap."""
    qs_full = np.ascontiguousarray(
        np.broadcast_to((np.float32(0.01) * dec["qs"])[:, None, None],
                        (N_AUG, P, 1)).astype(np.float32))
    in_maps = []
    overflow = []
    NCH = D // W
    x16 = x.astype(np.float16)
    for c in range(N_CORES):
        rows = slice(c * R, (c + 1) * R)
        # packed per-chunk stream, all augs: [gd f16 bytes | nz i8 bytes] x A
        gn = np.empty((R, NCH, N_AUG * 3 * W), np.uint8)
        gnv = gn.reshape(R, NCH, N_AUG, 3 * W)
        for a in range(N_AUG):
            gd_a = (dec["keep"][a][rows] *
                    (dec["gscale"][a] / dec["qs"][a])[None, :]).astype(
                        np.float16)
            gnv[:, :, a, :2 * W] = \
                gd_a.reshape(R, NCH, W).view(np.uint8).transpose(0, 1, 2)
            gnv[:, :, a, 2 * W:] = \
                dec["noise_i8"][a][rows].reshape(R, NCH, W).view(np.uint8)

        xq = np.zeros((T, P, D), np.float16)
        diag = np.zeros((N_AUG, T, P, P), np.float32)
        sp = np.zeros((N_AUG, T, P, P), np.float32)
        ar = np.arange(P)
        for t in range(T):
            slot_of = {}          # parent row j -> slot
            slot_js = []
            for a in range(N_AUG):
                base = c * R + t * P
                diag[a, t, ar, ar] = dec["c1"][a, base:base + P]
                mix_p = np.nonzero(dec["do_mix"][a, base:base + P])[0]
                for p in mix_p:
                    j = int(dec["j_sel"][a, base + p])
                    k = slot_of.get(j)
                    if k is None:
                        if len(slot_js) >= P:
                            overflow.append((a, base + p))
                            continue
                        k = len(slot_js)
                        slot_of[j] = k
                        slot_js.append(j)
                    sp[a, t, k, p] = dec["c2"][a, base + p]
            if slot_js:
                xq[t, :len(slot_js)] = x16[np.asarray(slot_js)]

        in_maps.append({
            "xb": x16[rows], "xq": xq, "gn": gn,
            "diag16": diag.astype(np.float16),
            "sp16": sp.astype(np.float16),
            "c1": np.ascontiguousarray(
                dec["c1"][:, rows].reshape(N_AUG, T, P, 1)),
            "qs": qs_full,
        })
    return in_maps, overflow


def _patch_overflow(counts, x, dec, overflow):
    """Host-exact recompute of rows whose mixup parent had no xq slot."""
    for a, i in overflow:
        gd_row = (dec["keep"][a][i] * dec["gscale"][a]).astype(np.float16)
        s = dec["c1"][a, i] * x[i] + dec["c2"][a, i] * x[dec["j_sel"][a, i]]
        h = gd_row.astype(np.float32) * s + \
            dec["qs"][a] * dec["noise_i8"][a][i].astype(np.float32)
        counts[a * B + i] = np.maximum(np.float32(0.01) * h, 0.0)


# ----------------------------------------------------------------------------
# device program (v3)
# ----------------------------------------------------------------------------
def _build_program():
    import concourse.bacc as bacc
    from concourse import mybir
    from concourse.tile import TileContext

    A = N_AUG
    nc = bacc.Bacc("TRN2")
    f32, f16, i8 = mybir.dt.float32, mybir.dt.float16, mybir.dt.int8
    t_xb = nc.dram_tensor("xb", [R, D], f16, kind="ExternalInput")
    t_xq = nc.dram_tensor("xq", [T, P, D], f16, kind="ExternalInput")
    NCHd = D // W
    t_gn = nc.dram_tensor("gn", [R, NCHd, A * 3 * W], mybir.dt.uint8,
                          kind="ExternalInput")
    t_c1 = nc.dram_tensor("c1", [A, T, P, 1], f32, kind="ExternalInput")
    t_diag = nc.dram_tensor("diag16", [A, T, P, P], f16, kind="ExternalInput")
    t_sp = nc.dram_tensor("sp16", [A, T, P, P], f16, kind="ExternalInput")
    t_qs = nc.dram_tensor("qs", [A, P, 1], f32, kind="ExternalInput")
    t_out = nc.dram_tensor("out", [A, R, D], f16, kind="ExternalOutput")

    Relu = mybir.ActivationFunctionType.Relu
    mult = mybir.AluOpType.mult
    add = mybir.AluOpType.add
    NCH = D // W
    NSUB = (W + BANK_F32 - 1) // BANK_F32
    WPAD = NSUB * BANK_F32

    with TileContext(nc) as tc:
        with (
            tc.tile_pool(name="wts", bufs=1) as wt_pool,
            tc.tile_pool(name="io", bufs=3) as io_pool,
            tc.tile_pool(name="io16", bufs=6) as io16_pool,
            tc.tile_pool(name="work", bufs=4) as work_pool,
            tc.tile_pool(name="psum", bufs=2, space="PSUM") as psum_pool,
        ):
            sp_sb, diag_sb, c1_sb = [], [], []
            for a in range(A):
                for t in range(T):
                    spt = wt_pool.tile([P, P], f16, name=f"sp_{a}_{t}")
                    nc.sync.dma_start(spt[:, :], t_sp[a, t])
                    dgt = wt_pool.tile([P, P], f16, name=f"diag_{a}_{t}")
                    nc.sync.dma_start(dgt[:, :], t_diag[a, t])
                    c1t = wt_pool.tile([P, 1], f32, name=f"c1_{a}_{t}")
                    nc.sync.dma_start(c1t[:, :], t_c1[a, t])
                    sp_sb.append(spt)
                    diag_sb.append(dgt)
                    c1_sb.append(c1t)
            qs_sb = []
            for a in range(A):
                qst = wt_pool.tile([P, 1], f32, name=f"qs_{a}")
                nc.sync.dma_start(qst[:, :], t_qs[a])
                qs_sb.append(qst)

            it = 0
            for t in range(T):
                rows = slice(t * P, (t + 1) * P)
                for ch in range(NCH):
                    cols = slice(ch * W, (ch + 1) * W)
                    xb_c = io_pool.tile([P, W], f16, name="xb_c", bufs=4)
                    nc.sync.dma_start(xb_c[:, :], t_xb[rows, cols])
                    xq_c = io_pool.tile([P, W], f16, name="xq_c", bufs=3)
                    nc.sync.dma_start(xq_c[:, :], t_xq[t, :, cols])
                    gn_c = io16_pool.tile([P, A * 3 * W], mybir.dt.uint8,
                                          name="gn_c", bufs=3)
                    nc.sync.dma_start(gn_c[:, :], t_gn[rows, ch])
                    for a in range(A):
                        gd_c = gn_c[:, a * 3 * W:a * 3 * W + 2 * W].bitcast(f16)
                        nz_c = gn_c[:, a * 3 * W + 2 * W:(a + 1) * 3 * W].bitcast(i8)

                        mix_on_pe = (it % 4 == 0)
                        it += 1
                        ps = psum_pool.tile([P, WPAD], f32, name="ps")
                        if mix_on_pe:
                            for s in range(NSUB):
                                lo = s * BANK_F32
                                hi = min(W, lo + BANK_F32)
                                nc.tensor.matmul(
                                    ps[:, lo:hi], diag_sb[a * T + t][:, :],
                                    xb_c[:, lo:hi], start=True, stop=False)
                            for s in range(NSUB):
                                lo = s * BANK_F32
                                hi = min(W, lo + BANK_F32)
                                nc.tensor.matmul(
                                    ps[:, lo:hi], sp_sb[a * T + t][:, :],
                                    xq_c[:, lo:hi], start=False, stop=True)
                        else:
                            for s in range(NSUB):
                                lo = s * BANK_F32
                                hi = min(W, lo + BANK_F32)
                                nc.tensor.matmul(
                                    ps[:, lo:hi], sp_sb[a * T + t][:, :],
                                    xq_c[:, lo:hi], start=True, stop=True)

                        sx = work_pool.tile([P, W], f32, name="sx")
                        if mix_on_pe:
                            # sx = psum * gd
                            nc.vector.tensor_tensor(sx[:, :], ps[:, 0:W],
                                                    gd_c, mult)
                        else:
                            # sx = c1*xb + psum, then *= gd
                            nc.vector.scalar_tensor_tensor(
                                sx[:, :], xb_c[:, :], c1_sb[a * T + t][:, :],
                                ps[:, 0:W], mult, add)
                            nc.vector.tensor_tensor(sx[:, :], sx[:, :],
                                                    gd_c, mult)
                        # sx += nz_i8 (dequant folded into gd and relu scale)
                        nc.gpsimd.tensor_tensor(sx[:, :], sx[:, :],
                                                nz_c, add)
                        out_c = io_pool.tile([P, W], f16, name="out_c", bufs=4)
                        nc.scalar.activation(out_c[:, :], sx[:, :], Relu,
                                             scale=qs_sb[a][:, :])
                        nc.scalar.dma_start(t_out[a, rows, cols], out_c[:, :])
    nc.finalize()
    return nc


def _get_program():
    key = (R, D, W, "v10")
    if key not in _PROGRAM_CACHE:
        _PROGRAM_CACHE[key] = _build_program()
    return _PROGRAM_CACHE[key]


# ----------------------------------------------------------------------------
# entry point
# ----------------------------------------------------------------------------
def kernel(x, ctx, y, cont_covs, cat_covs):
    from concourse.bass_utils import run_bass_kernel_spmd

    x = np.ascontiguousarray(np.asarray(x, dtype=np.float32))
    ctx = np.asarray(ctx)
    y = np.asarray(y)
    assert x.shape == (B, D), x.shape

    dec = _host_decompose(x, ctx, y)
    in_maps, overflow = _build_in_maps(x, dec)

    nc = _get_program()
    res = run_bass_kernel_spmd(nc, in_maps, core_ids=list(range(N_CORES)))

    counts = np.empty((N_AUG * B, D), np.float32)
    for c in range(N_CORES):
        out_c = res.results[c]["out"]
        for a in range(N_AUG):
            counts[a * B + c * R:(a * B + (c + 1) * R)] = \
                out_c[a].astype(np.float32)
    _patch_overflow(counts, x, dec, overflow)

    aug_ctxs = dec["ctxs"].reshape(-1)
    y_rep = np.tile(y, N_AUG)
    cont_rep = np.tile(np.asarray(cont_covs), (N_AUG, 1)).reshape(-1)
    cat_rep = np.tile(np.asarray(cat_covs), (N_AUG, 1)).reshape(-1)
    return counts, aug_ctxs, y_rep, cont_rep, cat_rep


# revision 18
# speedup vs baseline: 2.5128x; 1.3194x over previous
"""Trainium2 Bass kernel for nn_BatchAugmentation_53850299957920.

Reference pipeline (3 augs of x[2048, 20000]):
  same-class/cross-dataset mixup -> dropout -> per-row scale -> per-gene scale
  -> + 0.01*gaussian noise -> relu; concat augs (plus repeated ctx/y/cov
  metadata outputs).

Work split:
  * Host (exact jax-CPU threefry — RNG must match the reference bit-exactly
    for every discrete decision): all random draws, the [B,B] mixup candidate
    search, per-row/per-gene scalars.  The O(B*D) random tensors ship
    compactly: keep*gscale as f16, noise as int8 + per-aug dequant scale.
  * Device (8 NeuronCores, data-parallel over batch rows): all heavy [B,D]
    elementwise math:

      psum = diag(c1) @ xb_tile + Sp @ xq_tile                          (PE)
      sx   = psum * gd'                                                 (DVE)
      sx  += nz_i8                                                   (GPSIMD)
      out  = relu((0.01*qs) * sx)                                       (ACT)

    c1 = 100*scale*lam_eff, Sp[k,p] = 100*scale*(1-lam) for parent slot k ->
    row p, gd' = keep*gscale/qs (f16), nz_i8 = noise quantized to int8 with
    per-aug scale qs.  Folding 1/qs into gd and qs into the relu's
    per-partition scale makes the noise add a plain integer tensor_tensor.
    xq holds up to 128 deduplicated parent rows per 128-row tile (shared
    across the 3 augs); rare overflow rows are patched on the host.  The
    mix/noise work alternates between engines so no single engine gates the
    DMA-bound pipeline.
"""
import sys

if "/opt/trn_rl_repo" not in sys.path:
    sys.path.insert(0, "/opt/trn_rl_repo")

import numpy as np

N_AUG = 3
DROPOUT = 0.2
DS_MIN = 0.8
DS_MAX = 1.2
MIXUP_P = 0.3
ALPHA = 0.4
GENE_P = 0.2

N_CORES = 8
P = 128
B, D = 2048, 20000
R = B // N_CORES          # rows per core
T = R // P                # row-tiles per core
W = 2000                  # column chunk
BANK_F32 = 512            # one PSUM bank in f32 elements

_PROGRAM_CACHE = {}


# ----------------------------------------------------------------------------
# host side: exact RNG decomposition (jax CPU threefry == reference bits)
# ----------------------------------------------------------------------------
def _host_decompose(x, ctx, y):
    import jax
    import jax.numpy as jnp

    cpu = jax.devices("cpu")[0]
    Bx, Dx = x.shape
    base_key = jax.device_put(jax.random.key(42), cpu)

    c1 = np.empty((N_AUG, Bx), np.float32)
    c2 = np.empty((N_AUG, Bx), np.float32)
    j_sel_all = np.empty((N_AUG, Bx), np.int64)
    do_mix_all = np.empty((N_AUG, Bx), bool)
    ctxs = []
    keep_list = []
    gscale_list = []
    noise_list = []
    qs = np.empty((N_AUG,), np.float32)

    yy = np.asarray(y)
    cc = np.asarray(ctx)

    with jax.default_device(cpu):
        for a in range(N_AUG):
            k = jax.random.fold_in(base_key, a)
            k_mix, k_drop, k_scale, k_gmask, k_gscale, k_noise = \
                jax.random.split(k, 6)

            # --- mixup selection (bit-exact vs reference) ---
            k_sel, k_pick, k_lam, k_coin = jax.random.split(k_mix, 4)
            mix_mask = np.asarray(jax.random.uniform(k_sel, (Bx,))) < MIXUP_P
            cand = (yy[None, :] == yy[:, None]) & (cc[None, :] != cc[:, None])
            counts = cand.sum(axis=1).astype(np.int32)
            u = np.asarray(jax.random.uniform(k_pick, (Bx,)))
            kth = np.minimum(
                np.floor(u * counts.astype(np.float32)).astype(np.int32),
                np.maximum(counts - 1, 0),
            )
            csum = np.cumsum(cand.astype(np.int32), axis=1)
            j_sel = np.argmax(csum > kth[:, None], axis=1)
            lam = np.asarray(
                jax.random.beta(k_lam, ALPHA, ALPHA, (Bx,), dtype=jnp.float32)
            )
            do_mix = mix_mask & (counts > 0)
            coin = np.asarray(jax.random.uniform(k_coin, (Bx,))) < 0.5
            ctx_new = np.where(do_mix & (~coin), cc[j_sel], cc).astype(cc.dtype)

            scale = (
                np.asarray(jax.random.uniform(k_scale, (Bx, 1)))[:, 0]
                * (DS_MAX - DS_MIN) + DS_MIN
            ).astype(np.float32)
            lam_eff = np.where(do_mix, lam, np.float32(1.0)).astype(np.float32)
            c1[a] = 100.0 * scale * lam_eff
            c2[a] = np.where(do_mix, 100.0 * scale * (1.0 - lam_eff), 0.0)
            j_sel_all[a] = j_sel
            do_mix_all[a] = do_mix
            ctxs.append(ctx_new)

            gmask = np.asarray(jax.random.uniform(k_gmask, (Dx,))) < GENE_P
            gs_draw = np.asarray(jax.random.uniform(k_gscale, (Dx,))) * 0.4 + 0.8
            gscale_list.append(np.where(gmask, gs_draw, 1.0).astype(np.float32))

            # big exact draws
            keep_list.append(
                np.asarray(jax.random.uniform(k_drop, (Bx, Dx))) > DROPOUT
            )
            n = np.asarray(
                jax.random.normal(k_noise, (Bx, Dx), dtype=jnp.float32)
            )
            qs[a] = np.float32(np.abs(n).max() / 127.0)
            noise_list.append(
                np.clip(np.rint(n / qs[a]), -127, 127).astype(np.int8)
            )

    return dict(c1=c1, c2=c2, j_sel=j_sel_all, do_mix=do_mix_all,
                ctxs=np.stack(ctxs, axis=0), keep=keep_list,
                gscale=gscale_list, noise_i8=noise_list, qs=qs)


# ----------------------------------------------------------------------------
# per-core input assembly
# ----------------------------------------------------------------------------
def _build_in_maps(x, dec):
    """Returns (in_maps, overflow) where overflow is a list of
    (a, global_row) whose parent didn't fit in the 128-slot cap."""
    qs_full = np.ascontiguousarray(
        np.broadcast_to((np.float32(0.01) * dec["qs"])[:, None, None],
                        (N_AUG, P, 1)).astype(np.float32))
    in_maps = []
    overflow = []
    NCH = D // W
    x16 = x.astype(np.float16)
    for c in range(N_CORES):
        rows = slice(c * R, (c + 1) * R)
        # packed per-chunk stream, all augs: [gd f16 bytes | nz i8 bytes] x A
        gn = np.empty((R, NCH, N_AUG * 3 * W), np.uint8)
        gnv = gn.reshape(R, NCH, N_AUG, 3 * W)
        for a in range(N_AUG):
            gd_a = (dec["keep"][a][rows] *
                    (dec["gscale"][a] / dec["qs"][a])[None, :]).astype(
                        np.float16)
            gnv[:, :, a, :2 * W] = \
                gd_a.reshape(R, NCH, W).view(np.uint8).transpose(0, 1, 2)
            gnv[:, :, a, 2 * W:] = \
                dec["noise_i8"][a][rows].reshape(R, NCH, W).view(np.uint8)

        xq = np.zeros((T, P, D), np.float16)
        diag = np.zeros((N_AUG, T, P, P), np.float32)
        sp = np.zeros((N_AUG, T, P, P), np.float32)
        ar = np.arange(P)
        for t in range(T):
            slot_of = {}          # parent row j -> slot
            slot_js = []
            for a in range(N_AUG):
                base = c * R + t * P
                diag[a, t, ar, ar] = dec["c1"][a, base:base + P]
                mix_p = np.nonzero(dec["do_mix"][a, base:base + P])[0]
                for p in mix_p:
                    j = int(dec["j_sel"][a, base + p])
                    k = slot_of.get(j)
                    if k is None:
                        if len(slot_js) >= P:
                            overflow.append((a, base + p))
                            continue
                        k = len(slot_js)
                        slot_of[j] = k
                        slot_js.append(j)
                    sp[a, t, k, p] = dec["c2"][a, base + p]
            if slot_js:
                xq[t, :len(slot_js)] = x16[np.asarray(slot_js)]

        in_maps.append({
            "xb": x16[rows], "xq": xq, "gn": gn,
            "diag16": diag.astype(np.float16),
            "sp16": sp.astype(np.float16),
            "c1": np.ascontiguousarray(
                dec["c1"][:, rows].reshape(N_AUG, T, P, 1)),
            "qs": qs_full,
        })
    return in_maps, overflow


def _patch_overflow(counts, x, dec, overflow):
    """Host-exact recompute of rows whose mixup parent had no xq slot."""
    for a, i in overflow:
        gd_row = (dec["keep"][a][i] * dec["gscale"][a]).astype(np.float16)
        s = dec["c1"][a, i] * x[i] + dec["c2"][a, i] * x[dec["j_sel"][a, i]]
        h = gd_row.astype(np.float32) * s + \
            dec["qs"][a] * dec["noise_i8"][a][i].astype(np.float32)
        counts[a * B + i] = np.maximum(np.float32(0.01) * h, 0.0)


# ----------------------------------------------------------------------------
# device program (v3)
# ----------------------------------------------------------------------------
def _build_program():
    import concourse.bacc as bacc
    from concourse import mybir
    from concourse.tile import TileContext

    A = N_AUG
    nc = bacc.Bacc("TRN2")
    f32, f16, i8 = mybir.dt.float32, mybir.dt.float16, mybir.dt.int8
    t_xb = nc.dram_tensor("xb", [R, D], f16, kind="ExternalInput")
    t_xq = nc.dram_tensor("xq", [T, P, D], f16, kind="ExternalInput")
    NCHd = D // W
    t_gn = nc.dram_tensor("gn", [R, NCHd, A * 3 * W], mybir.dt.uint8,
                          kind="ExternalInput")
    t_c1 = nc.dram_tensor("c1", [A, T, P, 1], f32, kind="ExternalInput")
    t_diag = nc.dram_tensor("diag16", [A, T, P, P], f16, kind="ExternalInput")
    t_sp = nc.dram_tensor("sp16", [A, T, P, P], f16, kind="ExternalInput")
    t_qs = nc.dram_tensor("qs", [A, P, 1], f32, kind="ExternalInput")
    t_out = nc.dram_tensor("out", [A, R, D], f16, kind="ExternalOutput")

    Relu = mybir.ActivationFunctionType.Relu
    mult = mybir.AluOpType.mult
    add = mybir.AluOpType.add
    NCH = D // W
    NSUB = (W + BANK_F32 - 1) // BANK_F32
    WPAD = NSUB * BANK_F32

    with TileContext(nc) as tc:
        with (
            tc.tile_pool(name="wts", bufs=1) as wt_pool,
            tc.tile_pool(name="io", bufs=3) as io_pool,
            tc.tile_pool(name="io16", bufs=6) as io16_pool,
            tc.tile_pool(name="work", bufs=4) as work_pool,
            tc.tile_pool(name="psum", bufs=2, space="PSUM") as psum_pool,
        ):
            sp_sb, diag_sb, c1_sb = [], [], []
            for a in range(A):
                for t in range(T):
                    spt = wt_pool.tile([P, P], f16, name=f"sp_{a}_{t}")
                    nc.sync.dma_start(spt[:, :], t_sp[a, t])
                    dgt = wt_pool.tile([P, P], f16, name=f"diag_{a}_{t}")
                    nc.sync.dma_start(dgt[:, :], t_diag[a, t])
                    c1t = wt_pool.tile([P, 1], f32, name=f"c1_{a}_{t}")
                    nc.sync.dma_start(c1t[:, :], t_c1[a, t])
                    sp_sb.append(spt)
                    diag_sb.append(dgt)
                    c1_sb.append(c1t)
            qs_sb = []
            for a in range(A):
                qst = wt_pool.tile([P, 1], f32, name=f"qs_{a}")
                nc.sync.dma_start(qst[:, :], t_qs[a])
                qs_sb.append(qst)

            it = 0
            for t in range(T):
                rows = slice(t * P, (t + 1) * P)
                for ch in range(NCH):
                    cols = slice(ch * W, (ch + 1) * W)
                    xb_c = io_pool.tile([P, W], f16, name="xb_c", bufs=4)
                    nc.sync.dma_start(xb_c[:, :], t_xb[rows, cols])
                    xq_c = io_pool.tile([P, W], f16, name="xq_c", bufs=3)
                    nc.sync.dma_start(xq_c[:, :], t_xq[t, :, cols])
                    gn_c = io16_pool.tile([P, A * 3 * W], mybir.dt.uint8,
                                          name="gn_c", bufs=3)
                    nc.sync.dma_start(gn_c[:, :], t_gn[rows, ch])
                    for a in range(A):
                        gd_c = gn_c[:, a * 3 * W:a * 3 * W + 2 * W].bitcast(f16)
                        nz_c = gn_c[:, a * 3 * W + 2 * W:(a + 1) * 3 * W].bitcast(i8)

                        mix_on_pe = True
                        noise_on_dve = (it % 3 == 0)
                        it += 1
                        ps = psum_pool.tile([P, WPAD], f32, name="ps")
                        if mix_on_pe:
                            for s in range(NSUB):
                                lo = s * BANK_F32
                                hi = min(W, lo + BANK_F32)
                                nc.tensor.matmul(
                                    ps[:, lo:hi], diag_sb[a * T + t][:, :],
                                    xb_c[:, lo:hi], start=True, stop=False)
                            for s in range(NSUB):
                                lo = s * BANK_F32
                                hi = min(W, lo + BANK_F32)
                                nc.tensor.matmul(
                                    ps[:, lo:hi], sp_sb[a * T + t][:, :],
                                    xq_c[:, lo:hi], start=False, stop=True)
                        else:
                            for s in range(NSUB):
                                lo = s * BANK_F32
                                hi = min(W, lo + BANK_F32)
                                nc.tensor.matmul(
                                    ps[:, lo:hi], sp_sb[a * T + t][:, :],
                                    xq_c[:, lo:hi], start=True, stop=True)

                        sx = work_pool.tile([P, W], f32, name="sx")
                        if mix_on_pe:
                            # sx = psum * gd
                            nc.vector.tensor_tensor(sx[:, :], ps[:, 0:W],
                                                    gd_c, mult)
                        else:
                            # sx = c1*xb + psum, then *= gd
                            nc.vector.scalar_tensor_tensor(
                                sx[:, :], xb_c[:, :], c1_sb[a * T + t][:, :],
                                ps[:, 0:W], mult, add)
                            nc.vector.tensor_tensor(sx[:, :], sx[:, :],
                                                    gd_c, mult)
                        # sx += nz_i8 (dequant folded into gd and relu scale)
                        if noise_on_dve:
                            nc.vector.tensor_tensor(sx[:, :], sx[:, :],
                                                    nz_c, add)
                        else:
                            nc.gpsimd.tensor_tensor(sx[:, :], sx[:, :],
                                                    nz_c, add)
                        out_c = io_pool.tile([P, W], f16, name="out_c", bufs=4)
                        nc.scalar.activation(out_c[:, :], sx[:, :], Relu,
                                             scale=qs_sb[a][:, :])
                        nc.scalar.dma_start(t_out[a, rows, cols], out_c[:, :])
    nc.finalize()
    return nc


def _get_program():
    key = (R, D, W, "v11")
    if key not in _PROGRAM_CACHE:
        _PROGRAM_CACHE[key] = _build_program()
    return _PROGRAM_CACHE[key]


# ----------------------------------------------------------------------------
# entry point
# ----------------------------------------------------------------------------
def kernel(x, ctx, y, cont_covs, cat_covs):
    from concourse.bass_utils import run_bass_kernel_spmd

    x = np.ascontiguousarray(np.asarray(x, dtype=np.float32))
    ctx = np.asarray(ctx)
    y = np.asarray(y)
    assert x.shape == (B, D), x.shape

    dec = _host_decompose(x, ctx, y)
    in_maps, overflow = _build_in_maps(x, dec)

    nc = _get_program()
    res = run_bass_kernel_spmd(nc, in_maps, core_ids=list(range(N_CORES)))

    counts = np.empty((N_AUG * B, D), np.float32)
    for c in range(N_CORES):
        out_c = res.results[c]["out"]
        for a in range(N_AUG):
            counts[a * B + c * R:(a * B + (c + 1) * R)] = \
                out_c[a].astype(np.float32)
    _patch_overflow(counts, x, dec, overflow)

    aug_ctxs = dec["ctxs"].reshape(-1)
    y_rep = np.tile(y, N_AUG)
    cont_rep = np.tile(np.asarray(cont_covs), (N_AUG, 1)).reshape(-1)
    cat_rep = np.tile(np.asarray(cat_covs), (N_AUG, 1)).reshape(-1)
    return counts, aug_ctxs, y_rep, cont_rep, cat_rep


# revision 19
# speedup vs baseline: 2.5208x; 1.0032x over previous
"""Trainium2 Bass kernel for nn_BatchAugmentation_53850299957920.

Reference pipeline (3 augs of x[2048, 20000]):
  same-class/cross-dataset mixup -> dropout -> per-row scale -> per-gene scale
  -> + 0.01*gaussian noise -> relu; concat augs (plus repeated ctx/y/cov
  metadata outputs).

Work split:
  * Host (exact jax-CPU threefry — RNG must match the reference bit-exactly
    for every discrete decision): all random draws, the [B,B] mixup candidate
    search, per-row/per-gene scalars.  The O(B*D) random tensors ship
    compactly: keep*gscale as f16, noise as int8 + per-aug dequant scale.
  * Device (8 NeuronCores, data-parallel over batch rows): all heavy [B,D]
    elementwise math:

      psum = diag(c1) @ xb_tile + Sp @ xq_tile                          (PE)
      sx   = psum * gd'                                                 (DVE)
      sx  += nz_i8                                                   (GPSIMD)
      out  = relu((0.01*qs) * sx)                                       (ACT)

    c1 = 100*scale*lam_eff, Sp[k,p] = 100*scale*(1-lam) for parent slot k ->
    row p, gd' = keep*gscale/qs (f16), nz_i8 = noise quantized to int8 with
    per-aug scale qs.  Folding 1/qs into gd and qs into the relu's
    per-partition scale makes the noise add a plain integer tensor_tensor.
    xq holds up to 128 deduplicated parent rows per 128-row tile (shared
    across the 3 augs); rare overflow rows are patched on the host.  The
    mix/noise work alternates between engines so no single engine gates the
    DMA-bound pipeline.
"""
import sys

if "/opt/trn_rl_repo" not in sys.path:
    sys.path.insert(0, "/opt/trn_rl_repo")

import numpy as np

N_AUG = 3
DROPOUT = 0.2
DS_MIN = 0.8
DS_MAX = 1.2
MIXUP_P = 0.3
ALPHA = 0.4
GENE_P = 0.2

N_CORES = 8
P = 128
B, D = 2048, 20000
R = B // N_CORES          # rows per core
T = R // P                # row-tiles per core
W = 2000                  # column chunk
BANK_F32 = 512            # one PSUM bank in f32 elements

_PROGRAM_CACHE = {}


# ----------------------------------------------------------------------------
# host side: exact RNG decomposition (jax CPU threefry == reference bits)
# ----------------------------------------------------------------------------
def _host_decompose(x, ctx, y):
    import jax
    import jax.numpy as jnp

    cpu = jax.devices("cpu")[0]
    Bx, Dx = x.shape
    base_key = jax.device_put(jax.random.key(42), cpu)

    c1 = np.empty((N_AUG, Bx), np.float32)
    c2 = np.empty((N_AUG, Bx), np.float32)
    j_sel_all = np.empty((N_AUG, Bx), np.int64)
    do_mix_all = np.empty((N_AUG, Bx), bool)
    ctxs = []
    keep_list = []
    gscale_list = []
    noise_list = []
    qs = np.empty((N_AUG,), np.float32)

    yy = np.asarray(y)
    cc = np.asarray(ctx)

    with jax.default_device(cpu):
        for a in range(N_AUG):
            k = jax.random.fold_in(base_key, a)
            k_mix, k_drop, k_scale, k_gmask, k_gscale, k_noise = \
                jax.random.split(k, 6)

            # --- mixup selection (bit-exact vs reference) ---
            k_sel, k_pick, k_lam, k_coin = jax.random.split(k_mix, 4)
            mix_mask = np.asarray(jax.random.uniform(k_sel, (Bx,))) < MIXUP_P
            cand = (yy[None, :] == yy[:, None]) & (cc[None, :] != cc[:, None])
            counts = cand.sum(axis=1).astype(np.int32)
            u = np.asarray(jax.random.uniform(k_pick, (Bx,)))
            kth = np.minimum(
                np.floor(u * counts.astype(np.float32)).astype(np.int32),
                np.maximum(counts - 1, 0),
            )
            csum = np.cumsum(cand.astype(np.int32), axis=1)
            j_sel = np.argmax(csum > kth[:, None], axis=1)
            lam = np.asarray(
                jax.random.beta(k_lam, ALPHA, ALPHA, (Bx,), dtype=jnp.float32)
            )
            do_mix = mix_mask & (counts > 0)
            coin = np.asarray(jax.random.uniform(k_coin, (Bx,))) < 0.5
            ctx_new = np.where(do_mix & (~coin), cc[j_sel], cc).astype(cc.dtype)

            scale = (
                np.asarray(jax.random.uniform(k_scale, (Bx, 1)))[:, 0]
                * (DS_MAX - DS_MIN) + DS_MIN
            ).astype(np.float32)
            lam_eff = np.where(do_mix, lam, np.float32(1.0)).astype(np.float32)
            c1[a] = 100.0 * scale * lam_eff
            c2[a] = np.where(do_mix, 100.0 * scale * (1.0 - lam_eff), 0.0)
            j_sel_all[a] = j_sel
            do_mix_all[a] = do_mix
            ctxs.append(ctx_new)

            gmask = np.asarray(jax.random.uniform(k_gmask, (Dx,))) < GENE_P
            gs_draw = np.asarray(jax.random.uniform(k_gscale, (Dx,))) * 0.4 + 0.8
            gscale_list.append(np.where(gmask, gs_draw, 1.0).astype(np.float32))

            # big exact draws
            keep_list.append(
                np.asarray(jax.random.uniform(k_drop, (Bx, Dx))) > DROPOUT
            )
            n = np.asarray(
                jax.random.normal(k_noise, (Bx, Dx), dtype=jnp.float32)
            )
            qs[a] = np.float32(np.abs(n).max() / 127.0)
            noise_list.append(
                np.clip(np.rint(n / qs[a]), -127, 127).astype(np.int8)
            )

    return dict(c1=c1, c2=c2, j_sel=j_sel_all, do_mix=do_mix_all,
                ctxs=np.stack(ctxs, axis=0), keep=keep_list,
                gscale=gscale_list, noise_i8=noise_list, qs=qs)


# ----------------------------------------------------------------------------
# per-core input assembly
# ----------------------------------------------------------------------------
def _build_in_maps(x, dec):
    """Returns (in_maps, overflow) where overflow is a list of
    (a, global_row) whose parent didn't fit in the 128-slot cap."""
    qs_full = np.ascontiguousarray(
        np.broadcast_to((np.float32(0.01) * dec["qs"])[:, None, None],
                        (N_AUG, P, 1)).astype(np.float32))
    in_maps = []
    overflow = []
    NCH = D // W
    x16 = x.astype(np.float16)
    for c in range(N_CORES):
        rows = slice(c * R, (c + 1) * R)
        # packed per-chunk stream, all augs: [gd f16 bytes | nz i8 bytes] x A
        gn = np.empty((R, NCH, N_AUG * 3 * W), np.uint8)
        gnv = gn.reshape(R, NCH, N_AUG, 3 * W)
        for a in range(N_AUG):
            gd_a = (dec["keep"][a][rows] *
                    (dec["gscale"][a] / dec["qs"][a])[None, :]).astype(
                        np.float16)
            gnv[:, :, a, :2 * W] = \
                gd_a.reshape(R, NCH, W).view(np.uint8).transpose(0, 1, 2)
            gnv[:, :, a, 2 * W:] = \
                dec["noise_i8"][a][rows].reshape(R, NCH, W).view(np.uint8)

        xq = np.zeros((T, P, D), np.float16)
        diag = np.zeros((N_AUG, T, P, P), np.float32)
        sp = np.zeros((N_AUG, T, P, P), np.float32)
        ar = np.arange(P)
        for t in range(T):
            slot_of = {}          # parent row j -> slot
            slot_js = []
            for a in range(N_AUG):
                base = c * R + t * P
                diag[a, t, ar, ar] = dec["c1"][a, base:base + P]
                mix_p = np.nonzero(dec["do_mix"][a, base:base + P])[0]
                for p in mix_p:
                    j = int(dec["j_sel"][a, base + p])
                    k = slot_of.get(j)
                    if k is None:
                        if len(slot_js) >= P:
                            overflow.append((a, base + p))
                            continue
                        k = len(slot_js)
                        slot_of[j] = k
                        slot_js.append(j)
                    sp[a, t, k, p] = dec["c2"][a, base + p]
            if slot_js:
                xq[t, :len(slot_js)] = x16[np.asarray(slot_js)]

        xb16 = x16[rows]
        xbq = np.empty((T, P, NCH, 2 * W), np.float16)
        for t in range(T):
            xbq[t, :, :, :W] = \
                xb16[t * P:(t + 1) * P].reshape(P, NCH, W)
            xbq[t, :, :, W:] = xq[t].reshape(P, NCH, W)
        in_maps.append({
            "xbq": xbq, "gn": gn,
            "diag16": diag.astype(np.float16),
            "sp16": sp.astype(np.float16),
            "c1": np.ascontiguousarray(
                dec["c1"][:, rows].reshape(N_AUG, T, P, 1)),
            "qs": qs_full,
        })
    return in_maps, overflow


def _patch_overflow(counts, x, dec, overflow):
    """Host-exact recompute of rows whose mixup parent had no xq slot."""
    for a, i in overflow:
        gd_row = (dec["keep"][a][i] * dec["gscale"][a]).astype(np.float16)
        s = dec["c1"][a, i] * x[i] + dec["c2"][a, i] * x[dec["j_sel"][a, i]]
        h = gd_row.astype(np.float32) * s + \
            dec["qs"][a] * dec["noise_i8"][a][i].astype(np.float32)
        counts[a * B + i] = np.maximum(np.float32(0.01) * h, 0.0)


# ----------------------------------------------------------------------------
# device program (v3)
# ----------------------------------------------------------------------------
def _build_program():
    import concourse.bacc as bacc
    from concourse import mybir
    from concourse.tile import TileContext

    A = N_AUG
    nc = bacc.Bacc("TRN2")
    f32, f16, i8 = mybir.dt.float32, mybir.dt.float16, mybir.dt.int8
    NCHx = D // W
    t_xbq = nc.dram_tensor("xbq", [T, P, NCHx, 2 * W], f16,
                           kind="ExternalInput")
    NCHd = D // W
    t_gn = nc.dram_tensor("gn", [R, NCHd, A * 3 * W], mybir.dt.uint8,
                          kind="ExternalInput")
    t_c1 = nc.dram_tensor("c1", [A, T, P, 1], f32, kind="ExternalInput")
    t_diag = nc.dram_tensor("diag16", [A, T, P, P], f16, kind="ExternalInput")
    t_sp = nc.dram_tensor("sp16", [A, T, P, P], f16, kind="ExternalInput")
    t_qs = nc.dram_tensor("qs", [A, P, 1], f32, kind="ExternalInput")
    t_out = nc.dram_tensor("out", [R, NCHx, A * W], f16,
                           kind="ExternalOutput")

    Relu = mybir.ActivationFunctionType.Relu
    mult = mybir.AluOpType.mult
    add = mybir.AluOpType.add
    NCH = D // W
    NSUB = (W + BANK_F32 - 1) // BANK_F32
    WPAD = NSUB * BANK_F32

    with TileContext(nc) as tc:
        with (
            tc.tile_pool(name="wts", bufs=1) as wt_pool,
            tc.tile_pool(name="io", bufs=3) as io_pool,
            tc.tile_pool(name="io16", bufs=6) as io16_pool,
            tc.tile_pool(name="work", bufs=4) as work_pool,
            tc.tile_pool(name="psum", bufs=2, space="PSUM") as psum_pool,
        ):
            sp_sb, diag_sb, c1_sb = [], [], []
            for a in range(A):
                for t in range(T):
                    spt = wt_pool.tile([P, P], f16, name=f"sp_{a}_{t}")
                    nc.sync.dma_start(spt[:, :], t_sp[a, t])
                    dgt = wt_pool.tile([P, P], f16, name=f"diag_{a}_{t}")
                    nc.sync.dma_start(dgt[:, :], t_diag[a, t])
                    c1t = wt_pool.tile([P, 1], f32, name=f"c1_{a}_{t}")
                    nc.sync.dma_start(c1t[:, :], t_c1[a, t])
                    sp_sb.append(spt)
                    diag_sb.append(dgt)
                    c1_sb.append(c1t)
            qs_sb = []
            for a in range(A):
                qst = wt_pool.tile([P, 1], f32, name=f"qs_{a}")
                nc.sync.dma_start(qst[:, :], t_qs[a])
                qs_sb.append(qst)

            it = 0
            for t in range(T):
                rows = slice(t * P, (t + 1) * P)
                for ch in range(NCH):
                    cols = slice(ch * W, (ch + 1) * W)
                    xbq_c = io_pool.tile([P, 2 * W], f16, name="xbq_c",
                                         bufs=4)
                    nc.sync.dma_start(xbq_c[:, :], t_xbq[t, :, ch])
                    xb_c = xbq_c[:, 0:W]
                    xq_c = xbq_c[:, W:2 * W]
                    gn_c = io16_pool.tile([P, A * 3 * W], mybir.dt.uint8,
                                          name="gn_c", bufs=3)
                    nc.sync.dma_start(gn_c[:, :], t_gn[rows, ch])
                    out3 = io_pool.tile([P, A * W], f16, name="out3", bufs=3)
                    for a in range(A):
                        gd_c = gn_c[:, a * 3 * W:a * 3 * W + 2 * W].bitcast(f16)
                        nz_c = gn_c[:, a * 3 * W + 2 * W:(a + 1) * 3 * W].bitcast(i8)

                        mix_on_pe = True
                        noise_on_dve = (it % 3 == 0)
                        it += 1
                        ps = psum_pool.tile([P, WPAD], f32, name="ps")
                        if mix_on_pe:
                            for s in range(NSUB):
                                lo = s * BANK_F32
                                hi = min(W, lo + BANK_F32)
                                nc.tensor.matmul(
                                    ps[:, lo:hi], diag_sb[a * T + t][:, :],
                                    xb_c[:, lo:hi], start=True, stop=False)
                            for s in range(NSUB):
                                lo = s * BANK_F32
                                hi = min(W, lo + BANK_F32)
                                nc.tensor.matmul(
                                    ps[:, lo:hi], sp_sb[a * T + t][:, :],
                                    xq_c[:, lo:hi], start=False, stop=True)
                        else:
                            for s in range(NSUB):
                                lo = s * BANK_F32
                                hi = min(W, lo + BANK_F32)
                                nc.tensor.matmul(
                                    ps[:, lo:hi], sp_sb[a * T + t][:, :],
                                    xq_c[:, lo:hi], start=True, stop=True)

                        sx = work_pool.tile([P, W], f32, name="sx")
                        if mix_on_pe:
                            # sx = psum * gd
                            nc.vector.tensor_tensor(sx[:, :], ps[:, 0:W],
                                                    gd_c, mult)
                        else:
                            # sx = c1*xb + psum, then *= gd
                            nc.vector.scalar_tensor_tensor(
                                sx[:, :], xb_c, c1_sb[a * T + t][:, :],
                                ps[:, 0:W], mult, add)
                            nc.vector.tensor_tensor(sx[:, :], sx[:, :],
                                                    gd_c, mult)
                        # sx += nz_i8 (dequant folded into gd and relu scale)
                        if noise_on_dve:
                            nc.vector.tensor_tensor(sx[:, :], sx[:, :],
                                                    nz_c, add)
                        else:
                            nc.gpsimd.tensor_tensor(sx[:, :], sx[:, :],
                                                    nz_c, add)
                        nc.scalar.activation(out3[:, a * W:(a + 1) * W],
                                             sx[:, :], Relu,
                                             scale=qs_sb[a][:, :])
                    nc.scalar.dma_start(t_out[rows, ch], out3[:, :])
    nc.finalize()
    return nc


def _get_program():
    key = (R, D, W, "v12")
    if key not in _PROGRAM_CACHE:
        _PROGRAM_CACHE[key] = _build_program()
    return _PROGRAM_CACHE[key]


# ----------------------------------------------------------------------------
# entry point
# ----------------------------------------------------------------------------
def kernel(x, ctx, y, cont_covs, cat_covs):
    from concourse.bass_utils import run_bass_kernel_spmd

    x = np.ascontiguousarray(np.asarray(x, dtype=np.float32))
    ctx = np.asarray(ctx)
    y = np.asarray(y)
    assert x.shape == (B, D), x.shape

    dec = _host_decompose(x, ctx, y)
    in_maps, overflow = _build_in_maps(x, dec)

    nc = _get_program()
    res = run_bass_kernel_spmd(nc, in_maps, core_ids=list(range(N_CORES)))

    counts = np.empty((N_AUG * B, D), np.float32)
    for c in range(N_CORES):
        o = res.results[c]["out"].reshape(R, D // W, N_AUG, W)
        for a in range(N_AUG):
            counts[a * B + c * R:(a * B + (c + 1) * R)] = \
                o[:, :, a, :].reshape(R, D).astype(np.float32)
    _patch_overflow(counts, x, dec, overflow)

    aug_ctxs = dec["ctxs"].reshape(-1)
    y_rep = np.tile(y, N_AUG)
    cont_rep = np.tile(np.asarray(cont_covs), (N_AUG, 1)).reshape(-1)
    cat_rep = np.tile(np.asarray(cat_covs), (N_AUG, 1)).reshape(-1)
    return counts, aug_ctxs, y_rep, cont_rep, cat_rep
